# revision 2
# baseline (speedup 1.0000x reference)
"""Trainium2 Bass kernel for the 2-layer GAT + mean-pool + MLP head problem.

Strategy (8-core SPMD, single NEFF):
  - Nodes are sharded by destination across 8 cores (6250 each, padded 6272).
    Per-core local node l -> (block t = l % 49, lane p = l // 49); padded node
    table row r = core*6272 + p*49 + t so the SBUF->DRAM table write is
    contiguous per partition.
  - Per layer: each core computes an fp16 "aug" row [h | asrc | adst] (144
    cols) for its own nodes with one matmul per block (lhsT = x^T tile,
    rhs = [W | W@Asrc_bd | W@Adst_bd]); AllGather builds the full 50176-row
    gather table in every core HBM.  Pad rows get asrc/adst = -30000 so any
    edge slot pointing at them contributes exp(leaky(-3e4)) = 0 exactly.
  - Edge phase: REAL edges (self-loops are handled densely in the epilogue)
    are sorted by dst block and padded to T_b tiles of 128 edges per block
    (T_b = per-block max over cores; pad slots point at a local pad row so
    they vanish via the -30000 trick).  For batches of U tiles one indirect
    DMA per tile row gathers 768B src pair-rows [h_e|h_o|asrc_e|asrc_o|pad]
    and a second gathers 256B dst pair-rows for adst.
    ex = exp(max(z, 0.2z)) with z = asrc+adst; h_scaled = h*ex (broadcast
    per head); a one-hot [128e,128d] built by is_equal against an iota
    constant feeds matmul psum += onehot^T @ [h_scaled | ex], giving the
    unnormalized aggregation and the softmax denominators in one pass.
  - Block epilogue: the self-loop term exp(leaky(asrc+adst))*[h|1] is added
    from the resident local aug rows, then out = num * (1/max(s,1e-30)) per
    head, + bias, ELU; layer 1 feeds a PE transpose + matmul producing the
    next layer's aug rows; layer 2 feeds the graph-mean-pool matmul
    (device-built graph one-hot).
  - Pool partials are AllReduced (32KB), then every core runs the tiny MLP +
    log_softmax redundantly; core 0's packed [128,10] output is returned.

Wire format (the axon tunnel is ~45MB/s, so per-call H2D transfer dominates
the steady-state call time; everything below exists to shrink it).  Per core
we ship ONE int16 blob [128, W_ALL]:
  - x^T quantized to 6-bit ints with a per-node f16 scale (packed 8 values
    per 3 int16 words; 0.75B/elem vs 1B for fp8, final rel err ~7e-3 vs the
    2e-2 gate).  Unpacked on device with pure f32 arithmetic (the DVE has no
    int mod: floor(v/2^k) is computed as round-via-+2^23 plus an is_ge fix).
    The per-node scale is folded into the layer-1 aug matmul output.
  - per-node 6-bit scales (f16) and graph ids (u8 pairs).
  - edge streams at 3B/edge: an int16 whose low 15 bits are the src pair-row
    and sign bit is the src parity, plus a u8 (dst lane | dst parity << 7)
    packed in pairs.  Both gather index streams (src + dst) are derived and
    wrap-shuffled on device.
  - GAT weights and the MLP tail are NOT replicated on the wire: each core
    carries 1/8 of [w1aug|w2aug] (72 of 576 f16 cols) and 1/8 of the f32
    tail; two tiny on-device AllGathers reassemble them.
Iotas, identities, one-hots and parity masks are built on device.

The execution path bypasses run_bass_kernel_spmd's per-call re-jit: the
shard_map'd PJRT callable is built once and cached, so a steady-state call
is exactly {concat inputs, H2D over the tunnel, NEFF exec, D2H of 40KB}.

kernel(**inputs) takes the FULL unsharded inputs and returns
(log_softmax(logits), logits) like the reference.
"""

import numpy as np

import jax

# Persistent compilation cache: the per-call XLA+NEFF pipeline is ~0.7s of
# pure recompilation of an identical module otherwise.
jax.config.update("jax_compilation_cache_dir", "/tmp/jax_bass_cache")
jax.config.update("jax_persistent_cache_min_compile_time_secs", 0)
jax.config.update("jax_persistent_cache_min_entry_size_bytes", 0)

from jax.experimental.shard_map import shard_map
from jax.sharding import Mesh, PartitionSpec

import concourse.bass as bass
import concourse.mybir as mybir
import concourse.tile as tile
from concourse import bacc
from concourse.bass2jax import (_bass_exec_p, install_neuronx_cc_hook,
                                partition_id_tensor)

F16 = mybir.dt.float16
F32 = mybir.dt.float32
I16 = mybir.dt.int16
AX = mybir.AluOpType

NCORES = 8
HPW = 384  # h-gather pair-row width in f16 (768B): [h_e|h_o|as_e|as_o|pad]
C23 = 8388608.0  # 2^23: (v + C23) - C23 rounds f32 v to an integer
PAD_A = -30000.0  # pad-row asrc/adst: exp(leaky(-3e4)) underflows to 0


def gat_config(N=50000, E=800000, F=128, H=8, C=16, G=64, NCLS=10, U=24):
    NPC = N // NCORES
    BLOCKS = (NPC + 127) // 128
    NPAD = BLOCKS * 128
    return dict(N=N, E=E, F=F, H=H, C=C, G=G, NCLS=NCLS, U=U, NPC=NPC,
                BLOCKS=BLOCKS, NPAD=NPAD, TBLROWS=NCORES * NPAD, AUGW=F + 2 * H)


def _blockdiag(a, H, C):
    m = np.zeros((H * C, H), np.float32)
    for h in range(H):
        m[h * C:(h + 1) * C, h] = a[h]
    return m


def _offsets(meta):
    """int16-col offsets of the packed per-core blob (f32 regions 4B-aligned)."""
    NPAD, BLOCKS, NT = meta["NPAD"], meta["BLOCKS"], meta["NT"]
    o = {}
    o["X6W"] = (NPAD + 2) // 3  # base-40 packed: 3 values per int16 word
    o["PKW"] = (NT + 1) // 2
    o["GIDW"] = (BLOCKS + 1) // 2
    o["WSH"] = 2 * meta["AUGW"] // NCORES        # 72 f16 cols
    o["TLW"] = 4                                  # f32 cols per tail shard
    p = 0
    o["OX6"] = p; p += o["X6W"]
    o["OSC"] = p; p += BLOCKS
    o["OSIDX"] = p; p += NT
    o["OPK2"] = p; p += o["PKW"]
    o["OGID2"] = p; p += o["GIDW"]
    o["OWSH"] = p; p += o["WSH"]
    p += p % 2
    o["OTL"] = p; p += 2 * o["TLW"]
    o["OCN"] = p; p += 2
    o["W_ALL"] = p + p % 2
    return o


def host_prep(inputs, cfg):
    """Builds per-core device input dicts + meta. Pure index/layout work."""
    N, E, F, H, C, G = cfg["N"], cfg["E"], cfg["F"], cfg["H"], cfg["C"], cfg["G"]
    NPC, BLOCKS, NPAD = cfg["NPC"], cfg["BLOCKS"], cfg["NPAD"]
    AUGW = cfg["AUGW"]

    x = np.asarray(inputs["x"], np.float32)
    ei = np.asarray(inputs["edge_index"], np.int64)
    batch = np.asarray(inputs["batch"], np.int64)

    W1 = np.asarray(inputs["W1"], np.float32)
    W2 = np.asarray(inputs["W2"], np.float32)
    w1aug = np.concatenate(
        [W1, W1 @ _blockdiag(np.asarray(inputs["a_src1"], np.float32), H, C),
         W1 @ _blockdiag(np.asarray(inputs["a_dst1"], np.float32), H, C)], 1)
    w2aug = np.concatenate(
        [W2, W2 @ _blockdiag(np.asarray(inputs["a_src2"], np.float32), H, C),
         W2 @ _blockdiag(np.asarray(inputs["a_dst2"], np.float32), H, C)], 1)

    # self-loops are NOT streamed: they're added densely in the epilogue
    src = ei[0]
    dst = ei[1]

    core = dst // NPC
    loc = dst - core * NPC
    t_blk = loc % BLOCKS
    p_lane = loc // BLOCKS

    def g2r(g):
        c = g // NPC
        l = g - c * NPC
        return (c * NPAD + (l // BLOCKS) * BLOCKS + (l % BLOCKS)).astype(np.int32)

    key = (core * BLOCKS + t_blk).astype(np.int64)
    order = np.argsort(key, kind="stable")
    counts = np.bincount(key, minlength=NCORES * BLOCKS)
    # per-block tile count: max over cores (same program on all cores);
    # >=1 so every block's epilogue (incl. the self-loop term) runs
    TBS = np.maximum(
        np.ceil(counts.reshape(NCORES, BLOCKS).max(0) / 128).astype(int), 1)
    NT = int(TBS.sum())
    oft = np.concatenate([[0], np.cumsum(TBS)])  # tile offset per block

    src_rows = g2r(src[order])
    dst_rows = g2r(dst[order])
    p_s = p_lane[order]

    # pad slots point at a guaranteed-pad row on the own core (asrc=-3e4
    # there kills them: ex = 0) with dst lane 0 / parity 0 (harmless).
    pad_local = 127 * BLOCKS + (BLOCKS - 1)
    assert pad_local >= NPC, "lane-127/last-block row must be a pad row"
    srcR = np.zeros((NCORES, NT * 128), np.int32)
    dstR = np.zeros((NCORES, NT * 128), np.int32)
    dstloc = np.zeros((NCORES, NT * 128), np.int32)
    ofs = np.concatenate([[0], np.cumsum(counts)])
    for c in range(NCORES):
        srcR[c, :] = c * NPAD + pad_local
        for b in range(BLOCKS):
            k = c * BLOCKS + b
            cnt = counts[k]
            sl = slice(ofs[k], ofs[k + 1])
            s0 = oft[b] * 128
            srcR[c, s0:s0 + cnt] = src_rows[sl]
            dstR[c, s0:s0 + cnt] = dst_rows[sl]
            dstloc[c, s0:s0 + cnt] = p_s[sl]

    # src stream: int16 with low 15 bits = src pair row, sign bit = parity
    sidx_u = ((srcR >> 1) | ((srcR & 1) << 15)).astype(np.uint16)
    sidxT = np.ascontiguousarray(
        sidx_u.reshape(NCORES, NT, 128).transpose(0, 2, 1)).view(np.int16)

    # dst payload: u8 = lane | parity<<7, packed 2 tiles per int16 word
    pk8 = (dstloc + 128 * (dstR % 2)).astype(np.uint16)
    pkT8 = pk8.reshape(NCORES, NT, 128).transpose(0, 2, 1)  # [NC,128,NT]
    NT2 = NT + NT % 2
    pkp = np.zeros((NCORES, 128, NT2), np.uint16)
    pkp[:, :, :NT] = pkT8
    pk2 = (pkp[:, :, 0::2] | (pkp[:, :, 1::2] << 8)).astype(np.uint16)

    # x^T per core in (t,p) column order: col j <- node c*NPC + (j%128)*BLOCKS
    # + j//128.  40-level quantization with a per-node f16 scale: levels
    # (k - 19.5) * s, s = absmax/19.5; 3 values per int16 word in base 40.
    # Pad nodes get scale 0 (their aug row is scale * psum = 0), so their
    # nonzero dequant values (min |q-19.5| = 0.5) are harmless.
    rs = np.abs(x).max(axis=1) / 19.5
    rs16 = np.maximum(rs, 1e-8).astype(np.float16)
    q_all = np.clip(np.round(x / rs16.astype(np.float32)[:, None] + 19.5),
                    0, 39).astype(np.int64)
    tt = np.arange(NPAD) // 128
    pp = np.arange(NPAD) % 128
    l_of_col = pp * BLOCKS + tt
    ok = l_of_col < NPC
    XQW = (NPAD + 2) // 3
    x6 = np.zeros((NCORES, F, XQW), np.uint16)
    scl = np.zeros((NCORES, 128, BLOCKS), np.float16)
    for c in range(NCORES):
        cols = np.where(ok, c * NPC + np.minimum(l_of_col, NPC - 1), 0)
        q = np.zeros((F, 3 * XQW), np.int64)
        q[:, :NPAD] = q_all[cols].T                           # pads: q=0
        x6[c] = (q[:, 0::3] + 40 * q[:, 1::3]
                 + 1600 * q[:, 2::3]).astype(np.uint16)
        # scale for node (lane p, block t); 0 for pad nodes
        l_g = np.arange(128)[:, None] * BLOCKS + np.arange(BLOCKS)[None, :]
        okg = l_g < NPC
        scl[c] = np.where(
            okg, rs16[c * NPC + np.minimum(l_g, NPC - 1)], np.float16(0.0))

    # graph id per (lane p, block t) node; 200 for pad; u8 packed in pairs
    l_g = np.arange(128)[:, None] * BLOCKS + np.arange(BLOCKS)[None, :]
    okg = l_g < NPC
    GIDW = (BLOCKS + 1) // 2
    gid2 = np.zeros((NCORES, 128, GIDW), np.uint16)
    for c in range(NCORES):
        g8 = np.where(okg, batch[c * NPC + np.minimum(l_g, NPC - 1)],
                      200).astype(np.uint16)
        gp = np.zeros((128, 2 * GIDW), np.uint16)
        gp[:, :BLOCKS] = g8
        gid2[c] = gp[:, 0::2] | (gp[:, 1::2] << 8)

    cnt = np.bincount(batch, minlength=G).astype(np.float32)
    inv_cnt = (1.0 / np.maximum(cnt, 1.0)).astype(np.float32)

    b1 = np.asarray(inputs["b1"], np.float32)
    b2 = np.asarray(inputs["b2"], np.float32)
    l1b = np.asarray(inputs["lin1_b"], np.float32)
    l2b = np.asarray(inputs["lin2_b"], np.float32)
    meta = dict(cfg, NT=NT, U=min(cfg["U"], NT), TBS=tuple(int(t) for t in TBS),
                OFT=tuple(int(t) for t in oft),
                bias1=bool(np.any(b1 != 0)), bias2=bool(np.any(b2 != 0)),
                lbias1=bool(np.any(l1b != 0)), lbias2=bool(np.any(l2b != 0)))
    o = _offsets(meta)

    # weights: each core carries 1/8 of [w1aug | w2aug] (AllGathered on dev)
    w16 = np.concatenate([w1aug, w2aug], 1).astype(np.float16)  # [128, 576]
    # shared f32 tail: lin1W (cols 0:16), inv_cnt (col 16, parts 0:64),
    # lin2W (cols 17:27, parts 0:16); sharded 4 cols per core
    t32 = np.zeros((128, 8 * o["TLW"]), np.float32)
    t32[:, 0:16] = np.asarray(inputs["lin1_W"], np.float32)
    t32[0:G, 16] = inv_cnt
    t32[0:16, 17:17 + cfg["NCLS"]] = np.asarray(inputs["lin2_W"], np.float32)

    in_maps = []
    for c in range(NCORES):
        md = np.zeros((128, o["W_ALL"]), np.int16)
        md[:, o["OX6"]:o["OX6"] + o["X6W"]] = x6[c].view(np.int16)
        md[:, o["OSC"]:o["OSC"] + BLOCKS] = scl[c].view(np.int16)
        md[:, o["OSIDX"]:o["OSIDX"] + NT] = sidxT[c]
        md[:, o["OPK2"]:o["OPK2"] + o["PKW"]] = pk2[c].view(np.int16)
        md[:, o["OGID2"]:o["OGID2"] + o["GIDW"]] = gid2[c].view(np.int16)
        md[:, o["OWSH"]:o["OWSH"] + o["WSH"]] = \
            w16[:, c * o["WSH"]:(c + 1) * o["WSH"]].view(np.int16)
        md[:, o["OTL"]:o["OTL"] + 2 * o["TLW"]] = \
            np.ascontiguousarray(
                t32[:, c * o["TLW"]:(c + 1) * o["TLW"]]).view(np.int16)
        md[:, o["OCN"]:o["OCN"] + 2] = \
            np.full((128, 1), c * NPAD, np.float32).view(np.int16)
        m = dict(md=md)
        if meta["bias1"]:
            m["b1rep"] = np.broadcast_to(b1.astype(np.float32), (128, F)).copy()
        if meta["bias2"]:
            m["b2rep"] = np.broadcast_to(b2.astype(np.float32), (128, F)).copy()
        if meta["lbias1"]:
            m["l1brep"] = np.broadcast_to(l1b, (G, l1b.shape[0])).copy()
        if meta["lbias2"]:
            m["l2brep"] = np.broadcast_to(l2b, (G, l2b.shape[0])).copy()
        in_maps.append(m)
    return meta, in_maps


def build_nc(meta):
    F, H, C, G, NCLS = meta["F"], meta["H"], meta["C"], meta["G"], meta["NCLS"]
    BLOCKS, NPAD, TBLROWS = meta["BLOCKS"], meta["NPAD"], meta["TBLROWS"]
    NPC = meta["NPC"]
    NT, U, AUGW, TBS = meta["NT"], meta["U"], meta["AUGW"], meta["TBS"]
    REPW = 2 * F + H  # matmul rhs width: [hE*exE | hO*exO | ex]
    o = _offsets(meta)
    W_ALL = o["W_ALL"]
    NT2 = o["PKW"] * 2
    # tile -> (block, k-within-block)
    tilemap = [(b, k) for b in range(BLOCKS) for k in range(TBS[b])]
    OFT = meta["OFT"]

    # 2 SWDGE queues: the h-gather and a-gather generate their descriptors
    # on separate queues so the Q7 descriptor generation (the edge-phase
    # bottleneck) for the two streams can overlap.
    nc = bacc.Bacc("TRN2", target_bir_lowering=False, debug=False,
                   num_devices=NCORES, num_swdge_queues=2)

    # --- I/O ---
    d_m = nc.dram_tensor("md", [128, W_ALL], I16, kind="ExternalInput")
    d_bias1 = (nc.dram_tensor("b1rep", [128, F], F32, kind="ExternalInput")
               if meta["bias1"] else None)
    d_bias2 = (nc.dram_tensor("b2rep", [128, F], F32, kind="ExternalInput")
               if meta["bias2"] else None)
    d_l1b = (nc.dram_tensor("l1brep", [G, C], F32, kind="ExternalInput")
             if meta["lbias1"] else None)
    d_l2b = (nc.dram_tensor("l2brep", [G, NCLS], F32, kind="ExternalInput")
             if meta["lbias2"] else None)
    d_out = nc.dram_tensor("out", [2 * G, NCLS], F32, kind="ExternalOutput")

    # --- internal DRAM (collectives + reformatted gather tables) ---
    aug_loc = [nc.dram_tensor(f"aug_loc{i}", [NPAD, AUGW], F16) for i in (1, 2)]
    table = [nc.dram_tensor(f"table{i}", [TBLROWS, AUGW], F16, addr_space="Shared")
             for i in (1, 2)]
    # hp: pair rows [h_e|h_o|as_e|as_o|pad] (768B); ap: pair rows with the
    # a slices at cols 48:64 (even) / 112:128 (odd) (256B)
    hp_tbl = [nc.dram_tensor(f"hp{i}", [TBLROWS // 2, HPW], F16) for i in (1, 2)]
    ap_tbl = [nc.dram_tensor(f"ap{i}", [TBLROWS // 2, 128], F16) for i in (1, 2)]
    wpart = nc.dram_tensor("wpart", [128, o["WSH"]], F16)
    wfull = nc.dram_tensor("wfull", [NCORES * 128, o["WSH"]], F16,
                           addr_space="Shared")
    tpart = nc.dram_tensor("tpart", [128, o["TLW"]], F32)
    tfull = nc.dram_tensor("tfull", [NCORES * 128, o["TLW"]], F32,
                           addr_space="Shared")
    pool_part = nc.dram_tensor("pool_part", [G, F], F32)
    pool_full = nc.dram_tensor("pool_full", [G, F], F32, addr_space="Shared")
    RG = [list(range(NCORES))]

    from contextlib import ExitStack
    with tile.TileContext(nc) as tc, ExitStack() as ctx:
        cpool = ctx.enter_context(tc.tile_pool(name="consts", bufs=1))
        # unpack scratch lives only until the streams are derived; its pool
        # is released before the big edge-phase pools are allocated below
        ustack = ExitStack()
        upool = ustack.enter_context(tc.tile_pool(name="unpk", bufs=1))
        psp = ctx.enter_context(tc.tile_pool(name="ps", bufs=3, space="PSUM"))
        pst = ctx.enter_context(tc.tile_pool(name="pst", bufs=2, space="PSUM"))
        psa = ctx.enter_context(tc.tile_pool(name="psa", bufs=2, space="PSUM"))
        psg = ctx.enter_context(tc.tile_pool(name="psg", bufs=1, space="PSUM"))

        # ================= f32 integer helpers (no DVE int mod) ============
        def f_floordiv(out, in_, tmp, tmp2, m):
            """out = floor(in_ / m) for 0 <= in_ < 2^23, m a power of two.
            round-to-int via +2^23 bias, then -1 + (v >= rnd) fixes any tie
            direction.  out/tmp/tmp2 must be distinct f32 APs; out may NOT
            alias in_."""
            nc.vector.tensor_scalar(out=tmp, in0=in_, scalar1=1.0 / m,
                                    scalar2=None, op0=AX.mult)
            nc.vector.tensor_scalar(out=tmp2, in0=tmp, scalar1=C23,
                                    scalar2=C23, op0=AX.add, op1=AX.subtract)
            nc.vector.tensor_tensor(out=tmp, in0=tmp, in1=tmp2, op=AX.subtract)
            nc.vector.tensor_scalar(out=tmp, in0=tmp, scalar1=0.0,
                                    scalar2=None, op0=AX.is_ge)
            nc.vector.scalar_tensor_tensor(out=out, in0=tmp2, scalar=-1.0,
                                           op0=AX.add, in1=tmp, op1=AX.add)

        # ---- load + unpack the base-40 x^T blob (chunked scratch) ----
        # word w = q0 + 40*q1 + 1600*q2, q in [0,40); xt value = q - 19.5.
        # floor(v/m) for arbitrary m: rnd via +2^23 bias on v*(1/m) (approx),
        # then the EXACT integer d = v - m*rnd picks the fix direction.
        XQW = o["X6W"]
        NPADX = 3 * XQW
        xt_sb = cpool.tile([F, NPADX], F16, tag="xt")
        scale_sb = cpool.tile([128, BLOCKS], F16, tag="scl")
        nc.sync.dma_start(out=scale_sb[:],
                          in_=d_m[:, o["OSC"]:o["OSC"] + BLOCKS].bitcast(F16))
        sclf_sb = cpool.tile([128, BLOCKS], F32, tag="sclf")
        nc.vector.tensor_copy(out=sclf_sb[:], in_=scale_sb[:])

        def f_floordiv_any(out, in_, tmp, tmp2, m):
            """out = floor(in_ / m) for 0 <= in_ < 2^16, any integer m."""
            nc.vector.tensor_scalar(out=tmp, in0=in_, scalar1=1.0 / m,
                                    scalar2=None, op0=AX.mult)
            nc.vector.tensor_scalar(out=tmp2, in0=tmp, scalar1=C23,
                                    scalar2=C23, op0=AX.add, op1=AX.subtract)
            nc.vector.scalar_tensor_tensor(out=tmp, in0=tmp2,
                                           scalar=-float(m), op0=AX.mult,
                                           in1=in_, op1=AX.add)
            nc.vector.tensor_scalar(out=tmp, in0=tmp, scalar1=0.0,
                                    scalar2=None, op0=AX.is_ge)
            nc.vector.scalar_tensor_tensor(out=out, in0=tmp2, scalar=-1.0,
                                           op0=AX.add, in1=tmp, op1=AX.add)

        NCHUNK = 4
        NGC = (XQW + NCHUNK - 1) // NCHUNK   # words per chunk
        xqc = upool.tile([128, NGC], I16, tag="xqc")
        wfu = upool.tile([128, NGC], F32, tag="wfu")
        tA = upool.tile([128, NGC], F32, tag="tA")
        tB = upool.tile([128, NGC], F32, tag="tB")
        tC = upool.tile([128, NGC], F32, tag="tC")
        tD = upool.tile([128, NGC], F32, tag="tD")

        for ch in range(NCHUNK):
            g0 = ch * NGC
            gn = min(NGC, XQW - g0)
            if gn <= 0:
                break
            nc.sync.dma_start(
                out=xqc[:, :gn],
                in_=d_m[:, o["OX6"] + g0:o["OX6"] + g0 + gn])
            w = wfu[:, :gn]
            nc.vector.tensor_copy(out=w, in_=xqc[:, :gn])
            nc.vector.tensor_scalar(out=tA[:, :gn], in0=w, scalar1=0.0,
                                    scalar2=None, op0=AX.is_lt)
            nc.vector.scalar_tensor_tensor(out=w, in0=tA[:, :gn],
                                           scalar=65536.0, op0=AX.mult,
                                           in1=w, op1=AX.add)
            x3 = xt_sb[:, 3 * g0:3 * (g0 + gn)].rearrange(
                "p (g three) -> p g three", three=3)
            wv = w.rearrange("p (g one) -> p g one", one=1)
            vA = tA[:, :gn].rearrange("p (g one) -> p g one", one=1)
            vB = tB[:, :gn].rearrange("p (g one) -> p g one", one=1)
            vC = tC[:, :gn].rearrange("p (g one) -> p g one", one=1)
            vD = tD[:, :gn].rearrange("p (g one) -> p g one", one=1)
            # q2 = floor(w/1600); r = w - 1600*q2
            f_floordiv_any(vA, wv, vC, vD, 1600)
            nc.vector.tensor_scalar(out=x3[:, :, 2:3], in0=vA, scalar1=19.5,
                                    scalar2=None, op0=AX.subtract)
            nc.vector.scalar_tensor_tensor(out=vB, in0=vA, scalar=-1600.0,
                                           op0=AX.mult, in1=wv, op1=AX.add)
            # q1 = floor(r/40); q0 = r - 40*q1
            f_floordiv_any(vA, vB, vC, vD, 40)
            nc.vector.tensor_scalar(out=x3[:, :, 1:2], in0=vA, scalar1=19.5,
                                    scalar2=None, op0=AX.subtract)
            nc.vector.scalar_tensor_tensor(out=vC, in0=vA, scalar=-40.0,
                                           op0=AX.mult, in1=vB, op1=AX.add)
            nc.vector.tensor_scalar(out=x3[:, :, 0:1], in0=vC, scalar1=19.5,
                                    scalar2=None, op0=AX.subtract)

        # ---- edge-stream unpack ----
        # src: int16 sidx -> f32; sign bit = parity, low 15 bits = pair row
        sidx_sb = upool.tile([128, NT], I16, tag="sidx")
        nc.sync.dma_start(out=sidx_sb[:],
                          in_=d_m[:, o["OSIDX"]:o["OSIDX"] + NT])
        sv = upool.tile([128, NT], F32, tag="sv")
        nc.vector.tensor_copy(out=sv[:], in_=sidx_sb[:])
        psrcf = upool.tile([128, NT], F32, tag="psrcf")
        nc.vector.tensor_scalar(out=psrcf[:], in0=sv[:], scalar1=0.0,
                                scalar2=None, op0=AX.is_lt)
        psrc_sb = cpool.tile([128, NT], F16, tag="psrc")
        qsrc_sb = cpool.tile([128, NT], F16, tag="qsrc")
        nc.vector.tensor_copy(out=psrc_sb[:], in_=psrcf[:])
        nc.vector.tensor_scalar(out=qsrc_sb[:], in0=psrc_sb[:], scalar1=-1.0,
                                scalar2=1.0, op0=AX.mult, op1=AX.add)
        # spos = sv + 32768*psrc (f32) = src pair row in [0, 32768)
        spos = upool.tile([128, NT], F32, tag="spos")
        nc.vector.scalar_tensor_tensor(out=spos[:], in0=psrcf[:],
                                       scalar=32768.0, op0=AX.mult,
                                       in1=sv[:], op1=AX.add)
        ph16s = upool.tile([128, NT], I16, tag="ph16s")
        nc.vector.tensor_copy(out=ph16s[:], in_=spos[:])

        # dst payload: u8 pairs -> pkf [128, NT2] f16, then lane/parity
        pk2_sb = upool.tile([128, o["PKW"]], I16, tag="pk2")
        nc.sync.dma_start(out=pk2_sb[:],
                          in_=d_m[:, o["OPK2"]:o["OPK2"] + o["PKW"]])
        pv = upool.tile([128, o["PKW"]], F32, tag="pv")
        nc.vector.tensor_copy(out=pv[:], in_=pk2_sb[:])
        pneg = upool.tile([128, o["PKW"]], F32, tag="pneg")
        nc.vector.tensor_scalar(out=pneg[:], in0=pv[:], scalar1=0.0,
                                scalar2=None, op0=AX.is_lt)
        nc.vector.scalar_tensor_tensor(out=pv[:], in0=pneg[:], scalar=65536.0,
                                       op0=AX.mult, in1=pv[:], op1=AX.add)
        phi = upool.tile([128, o["PKW"]], F32, tag="phi")
        pt2 = upool.tile([128, o["PKW"]], F32, tag="pt2")
        f_floordiv(phi[:].rearrange("p (u one) -> p u one", one=1),
                   pv[:].rearrange("p (u one) -> p u one", one=1),
                   pneg[:].rearrange("p (u one) -> p u one", one=1),
                   pt2[:].rearrange("p (u one) -> p u one", one=1),
                   256)
        pkf = cpool.tile([128, NT2], F16, tag="pkf")
        pk2v = pkf[:].rearrange("p (u two) -> p u two", two=2)
        nc.vector.scalar_tensor_tensor(
            out=pk2v[:, :, 0:1],
            in0=phi[:].rearrange("p (u one) -> p u one", one=1),
            scalar=-256.0, op0=AX.mult,
            in1=pv[:].rearrange("p (u one) -> p u one", one=1), op1=AX.add)
        nc.vector.tensor_copy(
            out=pk2v[:, :, 1:2],
            in_=phi[:].rearrange("p (u one) -> p u one", one=1))
        pdst_sb = cpool.tile([128, NT], F16, tag="pdst")
        dstl_sb = cpool.tile([128, NT], F16, tag="dstl")
        nc.vector.tensor_scalar(out=pdst_sb[:], in0=pkf[:, 0:NT],
                                scalar1=128.0, scalar2=None, op0=AX.is_ge)
        nc.vector.scalar_tensor_tensor(out=dstl_sb[:], in0=pdst_sb[:],
                                       scalar=-128.0, op0=AX.mult,
                                       in1=pkf[:, 0:NT], op1=AX.add)

        # gid: u8 pairs -> gid_sb [128, BLOCKS] f16
        gid2_sb = upool.tile([128, o["GIDW"]], I16, tag="gid2")
        nc.sync.dma_start(out=gid2_sb[:],
                          in_=d_m[:, o["OGID2"]:o["OGID2"] + o["GIDW"]])
        gv = upool.tile([128, o["GIDW"]], F32, tag="gv")
        nc.vector.tensor_copy(out=gv[:], in_=gid2_sb[:])
        gneg = upool.tile([128, o["GIDW"]], F32, tag="gneg")
        nc.vector.tensor_scalar(out=gneg[:], in0=gv[:], scalar1=0.0,
                                scalar2=None, op0=AX.is_lt)
        nc.vector.scalar_tensor_tensor(out=gv[:], in0=gneg[:], scalar=65536.0,
                                       op0=AX.mult, in1=gv[:], op1=AX.add)
        ghi = upool.tile([128, o["GIDW"]], F32, tag="ghi")
        gt2 = upool.tile([128, o["GIDW"]], F32, tag="gt2")
        f_floordiv(ghi[:].rearrange("p (u one) -> p u one", one=1),
                   gv[:].rearrange("p (u one) -> p u one", one=1),
                   gneg[:].rearrange("p (u one) -> p u one", one=1),
                   gt2[:].rearrange("p (u one) -> p u one", one=1),
                   256)
        gid_sb = cpool.tile([128, 2 * o["GIDW"]], F16, tag="gid")
        gid2v = gid_sb[:].rearrange("p (u two) -> p u two", two=2)
        nc.vector.scalar_tensor_tensor(
            out=gid2v[:, :, 0:1],
            in0=ghi[:].rearrange("p (u one) -> p u one", one=1),
            scalar=-256.0, op0=AX.mult,
            in1=gv[:].rearrange("p (u one) -> p u one", one=1), op1=AX.add)
        nc.vector.tensor_copy(
            out=gid2v[:, :, 1:2],
            in_=ghi[:].rearrange("p (u one) -> p u one", one=1))

        # src gather stream: wrap-shuffle ph16s into dma_gather's layout
        # ([16 partitions, NT*8]; element (r, u*8+q) = pair row of edge slot
        # (partition q*16+r, tile u)) while the unpack scratch is still live
        idxr = cpool.tile([128, NT * 16], I16, tag="idxr")
        wraps = idxr[0:16, 0:NT * 8].rearrange("r (u q) -> r u q", q=8)
        for q in range(8):
            nc.sync.dma_start(
                out=wraps[:, :, q:q + 1],
                in_=ph16s[q * 16:(q + 1) * 16, :].rearrange(
                    "p (u one) -> p u one", one=1))
        ustack.close()  # release the unpack scratch pool

        # edge-phase pools, allocated in the space the unpack scratch used
        gpool = ctx.enter_context(tc.tile_pool(name="gath", bufs=2))
        hpool = ctx.enter_context(tc.tile_pool(name="hsex", bufs=2))
        opool = ctx.enter_context(tc.tile_pool(name="oneh", bufs=2))
        zpool = ctx.enter_context(tc.tile_pool(name="zl", bufs=3))
        apool = ctx.enter_context(tc.tile_pool(name="adL", bufs=2))
        epool = ctx.enter_context(tc.tile_pool(name="epi", bufs=3))
        augp = ctx.enter_context(tc.tile_pool(name="augsb", bufs=2))

        # ---- AllGather the weight + tail shards ----
        nc.sync.dma_start(out=wpart[:, :],
                          in_=d_m[:, o["OWSH"]:o["OWSH"] + o["WSH"]].bitcast(F16))
        nc.gpsimd.collective_compute(
            "AllGather", AX.bypass, replica_groups=RG,
            ins=[wpart[:, :].opt()], outs=[wfull[:, :].opt()])
        w_sb = cpool.tile([128, 2 * AUGW], F16, tag="wsb")
        for c in range(NCORES):
            nc.sync.dma_start(out=w_sb[:, c * o["WSH"]:(c + 1) * o["WSH"]],
                              in_=wfull[c * 128:(c + 1) * 128, :])
        nc.sync.dma_start(
            out=tpart[:, :],
            in_=d_m[:, o["OTL"]:o["OTL"] + 2 * o["TLW"]].bitcast(F32))
        nc.gpsimd.collective_compute(
            "AllGather", AX.bypass, replica_groups=RG,
            ins=[tpart[:, :].opt()], outs=[tfull[:, :].opt()])
        t32_sb = cpool.tile([128, 8 * o["TLW"]], F32, tag="t32")
        for c in range(NCORES):
            nc.sync.dma_start(out=t32_sb[:, c * o["TLW"]:(c + 1) * o["TLW"]],
                              in_=tfull[c * 128:(c + 1) * 128, :])
        cn_sb = cpool.tile([128, 1], F32, tag="cn")
        nc.sync.dma_start(out=cn_sb[:],
                          in_=d_m[:, o["OCN"]:o["OCN"] + 2].bitcast(F32))

        bias1_sb = bias2_sb = l1b_sb = l2b_sb = None
        if d_bias1 is not None:
            bias1_sb = cpool.tile([128, F], F32, tag="b1")
            nc.sync.dma_start(out=bias1_sb[:], in_=d_bias1[:, :])
        if d_bias2 is not None:
            bias2_sb = cpool.tile([128, F], F32, tag="b2")
            nc.sync.dma_start(out=bias2_sb[:], in_=d_bias2[:, :])
        if d_l1b is not None:
            l1b_sb = cpool.tile([G, C], F32, tag="l1b")
            nc.sync.dma_start(out=l1b_sb[:], in_=d_l1b[:, :])
        if d_l2b is not None:
            l2b_sb = cpool.tile([G, NCLS], F32, tag="l2b")
            nc.sync.dma_start(out=l2b_sb[:], in_=d_l2b[:, :])

        # ---- device-built constants ----
        iota_sb = cpool.tile([128, U * 128], F16, tag="iota")
        nc.gpsimd.iota(out=iota_sb[:], pattern=[[0, U], [1, 128]], base=0,
                       channel_multiplier=0,
                       allow_small_or_imprecise_dtypes=True)
        # identities via two iotas + is_equal (no negative channel mult)
        idh_sb = cpool.tile([128, 128], F16, tag="idh")
        rowh = cpool.tile([128, 128], F16, tag="rowh")
        nc.gpsimd.iota(out=idh_sb[:], pattern=[[1, 128]], base=0,
                       channel_multiplier=0,
                       allow_small_or_imprecise_dtypes=True)
        nc.gpsimd.iota(out=rowh[:], pattern=[[0, 128]], base=0,
                       channel_multiplier=1,
                       allow_small_or_imprecise_dtypes=True)
        nc.vector.tensor_tensor(out=idh_sb[:], in0=idh_sb[:], in1=rowh[:],
                                op=AX.is_equal)
        idf_sb = cpool.tile([64, 64], F32, tag="idf")
        rowf = cpool.tile([64, 64], F32, tag="rowf")
        nc.gpsimd.iota(out=idf_sb[:], pattern=[[1, 64]], base=0,
                       channel_multiplier=0,
                       allow_small_or_imprecise_dtypes=True)
        nc.gpsimd.iota(out=rowf[:], pattern=[[0, 64]], base=0,
                       channel_multiplier=1,
                       allow_small_or_imprecise_dtypes=True)
        nc.vector.tensor_tensor(out=idf_sb[:], in0=idf_sb[:], in1=rowf[:],
                                op=AX.is_equal)
        # graph one-hot: gone[p, t*G+g] = (gid[p,t] == g)
        gone_sb = cpool.tile([128, BLOCKS * G], F16, tag="gone")
        nc.gpsimd.iota(out=gone_sb[:], pattern=[[0, BLOCKS], [1, G]], base=0,
                       channel_multiplier=0,
                       allow_small_or_imprecise_dtypes=True)
        nc.vector.tensor_tensor(
            out=gone_sb[:].rearrange("p (t g) -> p t g", g=G),
            in0=gone_sb[:].rearrange("p (t g) -> p t g", g=G),
            in1=gid_sb[:, 0:BLOCKS].to_broadcast([128, BLOCKS, G]),
            op=AX.is_equal)

        # ---- derive the dst gather stream on device ----
        # dst pair row = (c*NPAD + dstl*BLOCKS + b - pdst) / 2, computed in
        # f32 (exact), cast to i16 and wrap-shuffled into the dst half of
        # idxr (cols NT*8:NT*16).
        drow = cpool.tile([128, NT], F32, tag="drow")
        for b in range(BLOCKS):
            nc.vector.memset(drow[:, OFT[b]:OFT[b + 1]], float(b))
        nc.vector.scalar_tensor_tensor(out=drow[:], in0=dstl_sb[:],
                                       scalar=float(BLOCKS), op0=AX.mult,
                                       in1=drow[:], op1=AX.add)
        nc.vector.tensor_scalar(out=drow[:], in0=drow[:],
                                scalar1=cn_sb[:, 0:1], scalar2=None,
                                op0=AX.add)
        nc.vector.scalar_tensor_tensor(out=drow[:], in0=pdst_sb[:],
                                       scalar=-1.0, op0=AX.mult,
                                       in1=drow[:], op1=AX.add)
        nc.vector.tensor_scalar(out=drow[:], in0=drow[:], scalar1=0.5,
                                scalar2=float(TBLROWS // 2 - 1), op0=AX.mult,
                                op1=AX.min)
        ph16 = cpool.tile([128, NT], I16, tag="ph16")
        nc.vector.tensor_copy(out=ph16[:], in_=drow[:])
        wrapv = idxr[0:16, NT * 8:NT * 16].rearrange("r (u q) -> r u q", q=8)
        for q in range(8):
            nc.sync.dma_start(
                out=wrapv[:, :, q:q + 1],
                in_=ph16[q * 16:(q + 1) * 16, :].rearrange(
                    "p (u one) -> p u one", one=1))
        # replicate 16 -> 128 partitions (dma_gather wants the stream on
        # every 16-partition group)
        nc.sync.dma_start(out=idxr[16:32, :], in_=idxr[0:16, :])
        nc.sync.dma_start(out=idxr[32:64, :], in_=idxr[0:32, :])
        nc.sync.dma_start(out=idxr[64:128, :], in_=idxr[0:64, :])

        # pad-row mask: msk[p, t] = -3e4 where local node l = p*BLOCKS+t is a
        # pad (l >= NPC), else 0.  Adding it to asrc/adst makes any gather of
        # a pad row produce ex = exp(leaky(-3e4 + .)) = 0.
        lpos_sb = cpool.tile([128, BLOCKS], F32, tag="lpos")
        nc.gpsimd.iota(out=lpos_sb[:], pattern=[[1, BLOCKS]], base=0,
                       channel_multiplier=BLOCKS,
                       allow_small_or_imprecise_dtypes=True)
        msk_sb = cpool.tile([128, BLOCKS], F16, tag="msk")
        nc.vector.tensor_scalar(out=msk_sb[:], in0=lpos_sb[:],
                                scalar1=float(NPC), scalar2=PAD_A,
                                op0=AX.is_ge, op1=AX.mult)

        def mask_pad_aug(aug_sb):
            a3 = aug_sb[:].rearrange("p (t a) -> p t a", a=AUGW)
            nc.vector.tensor_tensor(
                out=a3[:, :, F:F + 2 * H], in0=a3[:, :, F:F + 2 * H],
                in1=msk_sb[:].to_broadcast([128, BLOCKS, 2 * H]), op=AX.add)

        def build_aug_from_xt():
            """aug rows for own nodes from resident x^T (6-bit ints); the
            per-node dequant scale is folded into the psum->sbuf copy."""
            aug_sb = augp.tile([128, BLOCKS * AUGW], F16, tag="augsb")
            for t in range(BLOCKS):
                ps = psa.tile([128, AUGW], F32, tag="psaug")
                nc.tensor.matmul(out=ps[:], lhsT=xt_sb[:, t * 128:(t + 1) * 128],
                                 rhs=w_sb[:, 0:AUGW],
                                 start=True, stop=True)
                nc.vector.tensor_scalar(
                    out=aug_sb[:, t * AUGW:(t + 1) * AUGW], in0=ps[:],
                    scalar1=sclf_sb[:, t:t + 1], scalar2=None, op0=AX.mult)
            return aug_sb

        def publish_table(aug_sb, which):
            dst = aug_loc[which]
            # DRAM rows r = p*BLOCKS + t  <=> view [(p t), f] -> [p, (t f)]
            nc.sync.dma_start(
                out=dst[:, :].rearrange("(p t) f -> p (t f)", t=BLOCKS),
                in_=aug_sb[:])
            nc.gpsimd.collective_compute(
                "AllGather", AX.bypass, replica_groups=RG,
                ins=[dst[:, :].opt()], outs=[table[which][:, :].opt()])
            # reformat into pair-row gather tables (DRAM->DRAM)
            t3 = table[which][:, :].rearrange("(g two) f -> g two f", two=2)
            nc.sync.dma_start(
                out=hp_tbl[which][:, 0:2 * F].rearrange(
                    "g (two f) -> g two f", two=2),
                in_=t3[:, :, 0:F])
            nc.sync.dma_start(
                out=hp_tbl[which][:, 2 * F:2 * F + 2 * H].rearrange(
                    "g (two a) -> g two a", two=2),
                in_=t3[:, :, F:F + H])
            # full 128-col rows (finite pad): cols 48:64 = a_even,
            # cols 112:128 = a_odd; 0:48/64:112 are h-tail junk
            nc.sync.dma_start(
                out=ap_tbl[which][:, :].rearrange("g (two j) -> g two j", two=2),
                in_=t3[:, :, F - 48:F + 2 * H])

        def elu_inplace(v_sb, width, out_tile):
            """out_tile(fp16) = elu(v_sb) = max(v,0) + min(exp(v)-1, 0)."""
            t_sb = epool.tile([128, width], F32, tag="elu_t")
            nc.scalar.activation(out=t_sb[:], in_=v_sb[:],
                                 func=mybir.ActivationFunctionType.Exp)
            nc.vector.tensor_scalar(out=t_sb[:], in0=t_sb[:], scalar1=1.0,
                                    scalar2=0.0, op0=AX.subtract, op1=AX.min)
            nc.vector.scalar_tensor_tensor(out=out_tile[:], in0=v_sb[:],
                                           scalar=0.0, op0=AX.max,
                                           in1=t_sb[:], op1=AX.add)

        def edge_phase(layer, aug_sb):
            """layer 0: consumes table[0], produces aug tile for table[1].
               layer 1: consumes table[1], accumulates pool psum.  aug_sb is
               the CURRENT layer's local aug tile (for the dense self-loop
               term).  Returns next aug tile (layer 0) or pool psum."""
            bias_sb = (bias1_sb, bias2_sb)[layer]
            if layer == 0:
                out_aug = augp.tile([128, BLOCKS * AUGW], F16, tag="augsb")
            else:
                pool_ps = psg.tile([G, F], F32, tag="poolps")

            hp, ap = hp_tbl[layer], ap_tbl[layer]
            nbatch = (NT + U - 1) // U
            ps_cur = None
            for bi in range(nbatch):
                u0 = bi * U
                ub = min(U, NT - u0)
                # bulk gathers: [h|asrc] pair-rows by src//2 (768B) and a
                # pair-rows by dst//2 (256B) — 2 descriptors per edge
                ghp = gpool.tile([128, U * HPW], F16, tag="g")
                nc.gpsimd.dma_gather(
                    out_ap=ghp[:, :ub * HPW].rearrange(
                        "p (u f) -> p u f", f=HPW),
                    in_ap=hp[:, :], idxs_ap=idxr[:, u0 * 8:(u0 + ub) * 8],
                    num_idxs=ub * 128, num_idxs_reg=ub * 128, elem_size=HPW,
                    single_packet=False)
                gap = apool.tile([128, U * 128], F16, tag="gap")
                nc.gpsimd.dma_gather(
                    out_ap=gap[:, :ub * 128].rearrange(
                        "p (u f) -> p u f", f=128),
                    in_ap=ap[:, :],
                    idxs_ap=idxr[:, NT * 8 + u0 * 8:NT * 8 + (u0 + ub) * 8],
                    num_idxs=ub * 128, num_idxs_reg=ub * 128,
                    elem_size=128, single_packet=False, queue_num=1)
                g3 = ghp[:, :ub * HPW].rearrange("p (u f) -> p u f", f=HPW)
                ga = gap[:, :ub * 128].rearrange("p (u f) -> p u f", f=128)

                # z = asrc[src] + adst[dst] with parity selection:
                #   asrc = ae + psrc*(ao-ae); adst = be + pdst*(bo-be)
                zl = zpool.tile([128, U * H], F16, tag="zl")
                tsel = zpool.tile([128, U * H], F16, tag="tsel")
                psB = psrc_sb[:, u0:u0 + ub].to_broadcast([128, ub, H])
                pdB = pdst_sb[:, u0:u0 + ub].to_broadcast([128, ub, H])
                t3 = tsel[:, :ub * H].rearrange("p (u h) -> p u h", h=H)
                z3 = zl[:, :ub * H].rearrange("p (u h) -> p u h", h=H)
                nc.vector.tensor_tensor(out=t3, in0=g3[:, :, 2 * F + H:2 * F + 2 * H],
                                        in1=g3[:, :, 2 * F:2 * F + H],
                                        op=AX.subtract)
                nc.vector.tensor_tensor(out=t3, in0=t3, in1=psB, op=AX.mult)
                nc.vector.tensor_tensor(out=z3, in0=t3,
                                        in1=g3[:, :, 2 * F:2 * F + H], op=AX.add)
                nc.vector.tensor_tensor(out=t3, in0=ga[:, :, 120:128],
                                        in1=ga[:, :, 56:64], op=AX.subtract)
                nc.vector.tensor_tensor(out=t3, in0=t3, in1=pdB, op=AX.mult)
                nc.vector.tensor_tensor(out=z3, in0=z3, in1=t3, op=AX.add)
                nc.vector.tensor_tensor(out=z3, in0=z3,
                                        in1=ga[:, :, 56:64], op=AX.add)
                zv = zl[:, :ub * H]
                nc.vector.scalar_tensor_tensor(
                    out=zv, in0=zv, scalar=0.2, op0=AX.mult, in1=zv, op1=AX.max)

                he = hpool.tile([128, U * REPW], F16, tag="he")
                he3 = he[:, :ub * REPW].rearrange("p (u f) -> p u f", f=REPW)
                nc.scalar.activation(
                    out=he3[:, :, 2 * F:2 * F + H],
                    in_=zl[:, :ub * H].rearrange("p (u h) -> p u h", h=H),
                    func=mybir.ActivationFunctionType.Exp)
                # parity-masked ex, folded into the h scaling: the even half is
                # scaled by ex*(1-psrc), the odd half by ex*psrc, so the wrong
                # parity contributes zero and the psum halves sum to the answer
                exE = zpool.tile([128, U * H], F16, tag="exE")
                exO = zpool.tile([128, U * H], F16, tag="exO")
                eE3 = exE[:, :ub * H].rearrange("p (u h) -> p u h", h=H)
                eO3 = exO[:, :ub * H].rearrange("p (u h) -> p u h", h=H)
                nc.vector.tensor_tensor(
                    out=eE3, in0=he3[:, :, 2 * F:2 * F + H],
                    in1=qsrc_sb[:, u0:u0 + ub].to_broadcast([128, ub, H]),
                    op=AX.mult)
                nc.vector.tensor_tensor(
                    out=eO3, in0=he3[:, :, 2 * F:2 * F + H], in1=psB,
                    op=AX.mult)
                nc.vector.tensor_tensor(
                    out=he3[:, :, 0:F].rearrange("p u (h c) -> p u h c", c=C),
                    in0=g3[:, :, 0:F].rearrange("p u (h c) -> p u h c", c=C),
                    in1=eE3.to_broadcast([128, ub, H, C]), op=AX.mult)
                nc.vector.tensor_tensor(
                    out=he3[:, :, F:2 * F].rearrange("p u (h c) -> p u h c", c=C),
                    in0=g3[:, :, F:2 * F].rearrange("p u (h c) -> p u h c", c=C),
                    in1=eO3.to_broadcast([128, ub, H, C]), op=AX.mult)

                oh = opool.tile([128, U * 128], F16, tag="oh")
                nc.vector.tensor_tensor(
                    out=oh[:, :ub * 128].rearrange("p (u j) -> p u j", j=128),
                    in0=iota_sb[:, :ub * 128].rearrange("p (u j) -> p u j", j=128),
                    in1=dstl_sb[:, u0:u0 + ub].to_broadcast([128, ub, 128]),
                    op=AX.is_equal)

                for u in range(ub):
                    t = u0 + u
                    b, k = tilemap[t]
                    if k == 0:
                        ps_cur = psp.tile([128, REPW], F32, tag="psblk")
                    nc.tensor.matmul(
                        out=ps_cur[:], lhsT=oh[:, u * 128:(u + 1) * 128],
                        rhs=he[:, u * REPW:(u + 1) * REPW],
                        start=(k == 0), stop=(k == TBS[b] - 1))
                    if k == TBS[b] - 1:
                        # ---- block epilogue ----
                        # dense self-loop term from the local aug rows:
                        # ex_self = exp(leaky(asrc+adst)), s += ex_self,
                        # num += h_local * ex_self
                        zs = epool.tile([128, H], F16, tag="zs")
                        nc.vector.tensor_tensor(
                            out=zs[:], in0=aug_sb[:, b * AUGW + F:b * AUGW + F + H],
                            in1=aug_sb[:, b * AUGW + F + H:b * AUGW + F + 2 * H],
                            op=AX.add)
                        nc.vector.scalar_tensor_tensor(
                            out=zs[:], in0=zs[:], scalar=0.2, op0=AX.mult,
                            in1=zs[:], op1=AX.max)
                        exs = epool.tile([128, H], F32, tag="exs")
                        nc.scalar.activation(out=exs[:], in_=zs[:],
                                             func=mybir.ActivationFunctionType.Exp)
                        s_sb = epool.tile([128, H], F32, tag="s")
                        nc.vector.tensor_tensor(out=s_sb[:], in0=exs[:],
                                                in1=ps_cur[:, 2 * F:2 * F + H],
                                                op=AX.add)
                        nc.vector.tensor_scalar(out=s_sb[:], in0=s_sb[:],
                                                scalar1=1e-30, scalar2=None,
                                                op0=AX.max)
                        r_sb = epool.tile([128, H], F32, tag="r")
                        nc.vector.reciprocal(out=r_sb[:], in_=s_sb[:])
                        hs_sb = epool.tile([128, F], F32, tag="hs")
                        nc.vector.tensor_tensor(
                            out=hs_sb[:].rearrange("p (h c) -> p h c", c=C),
                            in0=aug_sb[:, b * AUGW:b * AUGW + F].rearrange(
                                "p (h c) -> p h c", c=C),
                            in1=exs[:].to_broadcast([128, H, C]), op=AX.mult)
                        hc_sb = epool.tile([128, F], F32, tag="hc")
                        nc.vector.tensor_tensor(out=hc_sb[:], in0=hs_sb[:],
                                                in1=ps_cur[:, 0:F], op=AX.add)
                        nc.vector.tensor_tensor(out=hc_sb[:], in0=hc_sb[:],
                                                in1=ps_cur[:, F:2 * F], op=AX.add)
                        v_sb = epool.tile([128, F], F32, tag="v")
                        nc.vector.tensor_tensor(
                            out=v_sb[:].rearrange("p (h c) -> p h c", c=C),
                            in0=hc_sb[:].rearrange("p (h c) -> p h c", c=C),
                            in1=r_sb[:].to_broadcast([128, H, C]), op=AX.mult)
                        if bias_sb is not None:
                            nc.vector.tensor_tensor(out=v_sb[:], in0=v_sb[:],
                                                    in1=bias_sb[:], op=AX.add)
                        eo = epool.tile([128, F], F16, tag="eo")
                        elu_inplace(v_sb, F, eo)
                        if layer == 0:
                            trp = pst.tile([128, 128], F16, tag="trps")
                            nc.tensor.transpose(out=trp[:], in_=eo[:],
                                                identity=idh_sb[:])
                            trs = epool.tile([128, 128], F16, tag="trsb")
                            nc.vector.tensor_copy(out=trs[:], in_=trp[:])
                            ap2 = psa.tile([128, AUGW], F32, tag="psaug")
                            nc.tensor.matmul(out=ap2[:], lhsT=trs[:],
                                             rhs=w_sb[:, AUGW:2 * AUGW],
                                             start=True, stop=True)
                            nc.vector.tensor_copy(
                                out=out_aug[:, b * AUGW:(b + 1) * AUGW],
                                in_=ap2[:])
                        else:
                            nc.tensor.matmul(
                                out=pool_ps[:],
                                lhsT=gone_sb[:, b * G:(b + 1) * G],
                                rhs=eo[:], start=(b == 0), stop=(b == BLOCKS - 1))
            return out_aug if layer == 0 else pool_ps

        # ---------------- pipeline ----------------
        aug1_sb = build_aug_from_xt()
        mask_pad_aug(aug1_sb)
        publish_table(aug1_sb, 0)
        aug2_sb = edge_phase(0, aug1_sb)
        mask_pad_aug(aug2_sb)
        publish_table(aug2_sb, 1)
        pool_ps = edge_phase(1, aug2_sb)

        # pooling allreduce
        psum_sb = epool.tile([G, F], F32, tag="poolsb")
        nc.vector.tensor_copy(out=psum_sb[:], in_=pool_ps[:])
        nc.sync.dma_start(out=pool_part[:, :], in_=psum_sb[:])
        nc.gpsimd.collective_compute(
            "AllReduce", AX.add, replica_groups=RG,
            ins=[pool_part[:, :].opt()], outs=[pool_full[:, :].opt()])
        hg_sb = epool.tile([G, F], F32, tag="hg")
        nc.sync.dma_start(out=hg_sb[:], in_=pool_full[:, :])
        nc.vector.tensor_scalar(out=hg_sb[:], in0=hg_sb[:],
                                scalar1=t32_sb[0:G, 16:17], scalar2=None,
                                op0=AX.mult)

        # MLP: z1 = elu(hg @ lin1W + b); logits = z1 @ lin2W + b
        hgT_ps = pst.tile([F, G], F32, tag="trps")
        nc.tensor.transpose(out=hgT_ps[:], in_=hg_sb[:], identity=idf_sb[:G, :G])
        hgT_sb = epool.tile([F, G], F32, tag="hgTs")
        nc.vector.tensor_copy(out=hgT_sb[:], in_=hgT_ps[:])
        z1_ps = psa.tile([G, C], F32, tag="psaug")
        nc.tensor.matmul(out=z1_ps[:], lhsT=hgT_sb[:], rhs=t32_sb[:, 0:16],
                         start=True, stop=True)
        z1_sb = epool.tile([G, C], F32, tag="z1s")
        if l1b_sb is not None:
            nc.vector.tensor_tensor(out=z1_sb[:], in0=z1_ps[:], in1=l1b_sb[:],
                                    op=AX.add)
        else:
            nc.vector.tensor_copy(out=z1_sb[:], in_=z1_ps[:])
        z1e_sb = epool.tile([G, C], F32, tag="z1e")
        t1 = epool.tile([G, C], F32, tag="t1")
        nc.scalar.activation(out=t1[:], in_=z1_sb[:],
                             func=mybir.ActivationFunctionType.Exp)
        nc.vector.tensor_scalar(out=t1[:], in0=t1[:], scalar1=1.0, scalar2=0.0,
                                op0=AX.subtract, op1=AX.min)
        nc.vector.scalar_tensor_tensor(out=z1e_sb[:], in0=z1_sb[:], scalar=0.0,
                                       op0=AX.max, in1=t1[:], op1=AX.add)
        z1T_ps = pst.tile([C, G], F32, tag="trps")
        nc.tensor.transpose(out=z1T_ps[:], in_=z1e_sb[:], identity=idf_sb[:G, :G])
        z1T_sb = epool.tile([C, G], F32, tag="z1Ts")
        nc.vector.tensor_copy(out=z1T_sb[:], in_=z1T_ps[:])
        lg_ps = psa.tile([G, NCLS], F32, tag="psaug")
        nc.tensor.matmul(out=lg_ps[:], lhsT=z1T_sb[:],
                         rhs=t32_sb[0:16, 17:17 + NCLS],
                         start=True, stop=True)
        lg_sb = epool.tile([G, NCLS], F32, tag="lgs")
        if l2b_sb is not None:
            nc.vector.tensor_tensor(out=lg_sb[:], in0=lg_ps[:], in1=l2b_sb[:],
                                    op=AX.add)
        else:
            nc.vector.tensor_copy(out=lg_sb[:], in_=lg_ps[:])

        # log_softmax
        m_sb = epool.tile([G, 1], F32, tag="m")
        nc.vector.tensor_reduce(out=m_sb[:], in_=lg_sb[:],
                                axis=mybir.AxisListType.X, op=AX.max)
        nm_sb = epool.tile([G, 1], F32, tag="nm")
        nc.vector.tensor_scalar(out=nm_sb[:], in0=m_sb[:], scalar1=-1.0,
                                scalar2=None, op0=AX.mult)
        e_sb = epool.tile([G, NCLS], F32, tag="esm")
        ss_sb = epool.tile([G, 1], F32, tag="ss")
        nc.scalar.activation(out=e_sb[:], in_=lg_sb[:],
                             func=mybir.ActivationFunctionType.Exp,
                             bias=nm_sb[:, 0:1], accum_out=ss_sb[:, 0:1])
        ls_sb = epool.tile([G, 1], F32, tag="ls")
        nc.scalar.activation(out=ls_sb[:], in_=ss_sb[:],
                             func=mybir.ActivationFunctionType.Ln)
        lsm_sb = epool.tile([G, NCLS], F32, tag="lsm")
        nc.vector.tensor_scalar(out=lsm_sb[:], in0=lg_sb[:],
                                scalar1=m_sb[:, 0:1], scalar2=ls_sb[:, 0:1],
                                op0=AX.subtract, op1=AX.subtract)

        nc.sync.dma_start(out=d_out[0:G, :], in_=lsm_sb[:])
        nc.sync.dma_start(out=d_out[G:2 * G, :], in_=lg_sb[:])

    nc.compile()  # bacc register allocation / DCE / act-table loads
    # The module is immutable from here on; memoize its (deterministic)
    # serialization so jit lowering skips the re-serialization.
    _json = nc.to_json_bytes()
    nc.to_json_bytes = lambda: _json
    return nc


class _Exec:
    """One-time-built PJRT callable for the SPMD kernel.  A steady-state
    call is exactly: concat per-core inputs, H2D, NEFF exec, D2H."""

    def __init__(self, meta):
        nc = build_nc(meta)
        install_neuronx_cc_hook()
        partition_name = (nc.partition_id_tensor.name
                          if nc.partition_id_tensor else None)
        in_names, out_names, out_avals = [], [], []
        for alloc in nc.m.functions[0].allocations:
            if not isinstance(alloc, mybir.MemoryLocationSet):
                continue
            name = alloc.memorylocations[0].name
            if alloc.kind == "ExternalInput":
                if name != partition_name:
                    in_names.append(name)
            elif alloc.kind == "ExternalOutput":
                out_names.append(name)
                out_avals.append(jax.core.ShapedArray(
                    tuple(alloc.tensor_shape), mybir.dt.np(alloc.dtype)))
        n_params = len(in_names)
        in_names_all = in_names + out_names
        if partition_name is not None:
            in_names_all.append(partition_name)

        def _body(*args):
            operands = list(args)
            if partition_name is not None:
                operands.append(partition_id_tensor())
            return tuple(_bass_exec_p.bind(
                *operands, out_avals=tuple(out_avals),
                in_names=tuple(in_names_all), out_names=tuple(out_names),
                lowering_input_output_aliases=(), sim_require_finite=True,
                sim_require_nnan=True, nc=nc))

        devices = jax.devices()[:NCORES]
        mesh = Mesh(np.asarray(devices), ("core",))
        n_outs = len(out_names)
        self._fn = jax.jit(
            shard_map(_body, mesh=mesh,
                      in_specs=(PartitionSpec("core"),) * (n_params + n_outs),
                      out_specs=(PartitionSpec("core"),) * n_outs,
                      check_rep=False),
            keep_unused=True)
        self.in_names = in_names
        self.out_names = out_names
        self.out_avals = out_avals
        self._zeros = [np.zeros((NCORES * a.shape[0], *a.shape[1:]), a.dtype)
                       for a in out_avals]

    def __call__(self, in_maps):
        concat_in = [
            np.concatenate([np.asarray(m[name]) for m in in_maps], axis=0)
            for name in self.in_names]
        outs = self._fn(*concat_in, *self._zeros)
        return [
            {name: np.asarray(outs[i]).reshape(
                NCORES, *self.out_avals[i].shape)[c]
             for i, name in enumerate(self.out_names)}
            for c in range(NCORES)]


_EXEC_CACHE = {}


def get_exec(meta):
    key = (meta["N"], meta["E"], meta["NT"], meta["U"], meta["TBS"],
           meta["bias1"], meta["bias2"], meta["lbias1"], meta["lbias2"])
    if key not in _EXEC_CACHE:
        _EXEC_CACHE[key] = _Exec(meta)
    return _EXEC_CACHE[key]


def run_gat(inputs, cfg):
    meta, in_maps = host_prep(inputs, cfg)
    ex = get_exec(meta)
    results = ex(in_maps)
    G, NCLS = cfg["G"], cfg["NCLS"]
    out = results[0]["out"]
    return (out[0:G, :], out[G:2 * G, :]), (ex, in_maps)


def kernel(**inputs):
    (lsm, logits), _ = run_gat(inputs, gat_config())
    return lsm.astype(np.float32), logits.astype(np.float32)


# revision 6
# speedup vs baseline: 1.0120x; 1.0120x over previous
"""Trainium2 Bass kernel for the 2-layer GAT + mean-pool + MLP head problem.

Strategy (8-core SPMD, single NEFF):
  - Nodes are sharded by destination across 8 cores (6250 each, padded 6272).
    Per-core local node l -> (block t = l % 49, lane p = l // 49); padded node
    table row r = core*6272 + p*49 + t so the SBUF->DRAM table write is
    contiguous per partition.
  - Per layer: each core computes an fp16 "aug" row [h | asrc | adst] (144
    cols) for its own nodes with one matmul per block (lhsT = x^T tile,
    rhs = [W | W@Asrc_bd | W@Adst_bd]); AllGather builds the full 50176-row
    gather table in every core HBM.  Pad rows get asrc/adst = -30000 so any
    edge slot pointing at them contributes exp(leaky(-3e4)) = 0 exactly.
  - Edge phase: REAL edges (self-loops are handled densely in the epilogue)
    are sorted by dst block and padded to T_b tiles of 128 edges per block
    (T_b = per-block max over cores; pad slots point at a local pad row so
    they vanish via the -30000 trick).  For batches of U tiles one indirect
    DMA per tile row gathers 768B src pair-rows [h_e|h_o|asrc_e|asrc_o|pad]
    and a second gathers 256B dst pair-rows for adst.
    ex = exp(max(z, 0.2z)) with z = asrc+adst; h_scaled = h*ex (broadcast
    per head); a one-hot [128e,128d] built by is_equal against an iota
    constant feeds matmul psum += onehot^T @ [h_scaled | ex], giving the
    unnormalized aggregation and the softmax denominators in one pass.
  - Block epilogue: the self-loop term exp(leaky(asrc+adst))*[h|1] is added
    from the resident local aug rows, then out = num * (1/max(s,1e-30)) per
    head, + bias, ELU; layer 1 feeds a PE transpose + matmul producing the
    next layer's aug rows; layer 2 feeds the graph-mean-pool matmul
    (device-built graph one-hot).
  - Pool partials are AllReduced (32KB), then every core runs the tiny MLP +
    log_softmax redundantly; core 0's packed [128,10] output is returned.

Wire format (the axon tunnel is ~45MB/s, so per-call H2D transfer dominates
the steady-state call time; everything below exists to shrink it).  Per core
we ship ONE int16 blob [128, W_ALL]:
  - x^T quantized to 6-bit ints with a per-node f16 scale (packed 8 values
    per 3 int16 words; 0.75B/elem vs 1B for fp8, final rel err ~7e-3 vs the
    2e-2 gate).  Unpacked on device with pure f32 arithmetic (the DVE has no
    int mod: floor(v/2^k) is computed as round-via-+2^23 plus an is_ge fix).
    The per-node scale is folded into the layer-1 aug matmul output.
  - per-node 6-bit scales (f16) and graph ids (u8 pairs).
  - edge streams at 3B/edge: an int16 whose low 15 bits are the src pair-row
    and sign bit is the src parity, plus a u8 (dst lane | dst parity << 7)
    packed in pairs.  Both gather index streams (src + dst) are derived and
    wrap-shuffled on device.
  - GAT weights and the MLP tail are NOT replicated on the wire: each core
    carries 1/8 of [w1aug|w2aug] (72 of 576 f16 cols) and 1/8 of the f32
    tail; two tiny on-device AllGathers reassemble them.
Iotas, identities, one-hots and parity masks are built on device.

The execution path bypasses run_bass_kernel_spmd's per-call re-jit: the
shard_map'd PJRT callable is built once and cached, so a steady-state call
is exactly {concat inputs, H2D over the tunnel, NEFF exec, D2H of 40KB}.

kernel(**inputs) takes the FULL unsharded inputs and returns
(log_softmax(logits), logits) like the reference.
"""

import numpy as np

import jax

# Persistent compilation cache: the per-call XLA+NEFF pipeline is ~0.7s of
# pure recompilation of an identical module otherwise.
jax.config.update("jax_compilation_cache_dir", "/tmp/jax_bass_cache")
jax.config.update("jax_persistent_cache_min_compile_time_secs", 0)
jax.config.update("jax_persistent_cache_min_entry_size_bytes", 0)

from jax.experimental.shard_map import shard_map
from jax.sharding import Mesh, NamedSharding, PartitionSpec

import concourse.bass as bass
import concourse.mybir as mybir
import concourse.tile as tile
from concourse import bacc
from concourse.bass2jax import (_bass_exec_p, install_neuronx_cc_hook,
                                partition_id_tensor)

F16 = mybir.dt.float16
F32 = mybir.dt.float32
I16 = mybir.dt.int16
AX = mybir.AluOpType

NCORES = 8
HPW = 384  # h-gather pair-row width in f16 (768B): [h_e|h_o|as_e|as_o|pad]
C23 = 8388608.0  # 2^23: (v + C23) - C23 rounds f32 v to an integer
PAD_A = -30000.0  # pad-row asrc/adst: exp(leaky(-3e4)) underflows to 0


def gat_config(N=50000, E=800000, F=128, H=8, C=16, G=64, NCLS=10, U=24):
    NPC = N // NCORES
    BLOCKS = (NPC + 127) // 128
    NPAD = BLOCKS * 128
    return dict(N=N, E=E, F=F, H=H, C=C, G=G, NCLS=NCLS, U=U, NPC=NPC,
                BLOCKS=BLOCKS, NPAD=NPAD, TBLROWS=NCORES * NPAD, AUGW=F + 2 * H)


def _blockdiag(a, H, C):
    m = np.zeros((H * C, H), np.float32)
    for h in range(H):
        m[h * C:(h + 1) * C, h] = a[h]
    return m


def _offsets(meta):
    """int16-col offsets of the packed per-core blob (f32 regions 4B-aligned)."""
    NPAD, BLOCKS, NT = meta["NPAD"], meta["BLOCKS"], meta["NT"]
    o = {}
    o["X6W"] = (NPAD + 2) // 3  # base-40 packed: 3 values per int16 word
    o["PKW"] = (NT + 1) // 2
    o["GIDW"] = (BLOCKS + 1) // 2
    o["WSH"] = 2 * meta["AUGW"] // NCORES        # 72 f16 cols
    o["TLW"] = 4                                  # f32 cols per tail shard
    p = 0
    o["OX6"] = p; p += o["X6W"]
    o["OSC"] = p; p += BLOCKS
    o["OSIDX"] = p; p += NT
    o["OPK2"] = p; p += o["PKW"]
    o["OGID2"] = p; p += o["GIDW"]
    o["OWSH"] = p; p += o["WSH"]
    p += p % 2
    o["OTL"] = p; p += 2 * o["TLW"]
    o["OCN"] = p; p += 2
    o["W_ALL"] = p + p % 2
    return o


def host_prep(inputs, cfg):
    """Builds per-core device input dicts + meta. Pure index/layout work."""
    N, E, F, H, C, G = cfg["N"], cfg["E"], cfg["F"], cfg["H"], cfg["C"], cfg["G"]
    NPC, BLOCKS, NPAD = cfg["NPC"], cfg["BLOCKS"], cfg["NPAD"]
    AUGW = cfg["AUGW"]

    x = np.asarray(inputs["x"], np.float32)
    ei = np.asarray(inputs["edge_index"], np.int64)
    batch = np.asarray(inputs["batch"], np.int64)

    W1 = np.asarray(inputs["W1"], np.float32)
    W2 = np.asarray(inputs["W2"], np.float32)
    w1aug = np.concatenate(
        [W1, W1 @ _blockdiag(np.asarray(inputs["a_src1"], np.float32), H, C),
         W1 @ _blockdiag(np.asarray(inputs["a_dst1"], np.float32), H, C)], 1)
    w2aug = np.concatenate(
        [W2, W2 @ _blockdiag(np.asarray(inputs["a_src2"], np.float32), H, C),
         W2 @ _blockdiag(np.asarray(inputs["a_dst2"], np.float32), H, C)], 1)

    # self-loops are NOT streamed: they're added densely in the epilogue
    src = ei[0]
    dst = ei[1]

    core = dst // NPC
    loc = dst - core * NPC
    t_blk = loc % BLOCKS
    p_lane = loc // BLOCKS

    def g2r(g):
        c = g // NPC
        l = g - c * NPC
        return (c * NPAD + (l // BLOCKS) * BLOCKS + (l % BLOCKS)).astype(np.int32)

    key = (core * BLOCKS + t_blk).astype(np.int64)
    order = np.argsort(key, kind="stable")
    counts = np.bincount(key, minlength=NCORES * BLOCKS)
    # per-block tile count: max over cores (same program on all cores);
    # >=1 so every block's epilogue (incl. the self-loop term) runs
    TBS = np.maximum(
        np.ceil(counts.reshape(NCORES, BLOCKS).max(0) / 128).astype(int), 1)
    NT = int(TBS.sum())
    oft = np.concatenate([[0], np.cumsum(TBS)])  # tile offset per block

    src_rows = g2r(src[order])
    dst_rows = g2r(dst[order])
    p_s = p_lane[order]

    # pad slots point at a guaranteed-pad row on the own core (asrc=-3e4
    # there kills them: ex = 0) with dst lane 0 / parity 0 (harmless).
    pad_local = 127 * BLOCKS + (BLOCKS - 1)
    assert pad_local >= NPC, "lane-127/last-block row must be a pad row"
    srcR = np.zeros((NCORES, NT * 128), np.int32)
    dstR = np.zeros((NCORES, NT * 128), np.int32)
    dstloc = np.zeros((NCORES, NT * 128), np.int32)
    ofs = np.concatenate([[0], np.cumsum(counts)])
    for c in range(NCORES):
        srcR[c, :] = c * NPAD + pad_local
        for b in range(BLOCKS):
            k = c * BLOCKS + b
            cnt = counts[k]
            sl = slice(ofs[k], ofs[k + 1])
            s0 = oft[b] * 128
            srcR[c, s0:s0 + cnt] = src_rows[sl]
            dstR[c, s0:s0 + cnt] = dst_rows[sl]
            dstloc[c, s0:s0 + cnt] = p_s[sl]

    # src stream: int16 with low 15 bits = src pair row, sign bit = parity
    sidx_u = ((srcR >> 1) | ((srcR & 1) << 15)).astype(np.uint16)
    sidxT = np.ascontiguousarray(
        sidx_u.reshape(NCORES, NT, 128).transpose(0, 2, 1)).view(np.int16)

    # dst payload: u8 = lane | parity<<7, packed 2 tiles per int16 word
    pk8 = (dstloc + 128 * (dstR % 2)).astype(np.uint16)
    pkT8 = pk8.reshape(NCORES, NT, 128).transpose(0, 2, 1)  # [NC,128,NT]
    NT2 = NT + NT % 2
    pkp = np.zeros((NCORES, 128, NT2), np.uint16)
    pkp[:, :, :NT] = pkT8
    pk2 = (pkp[:, :, 0::2] | (pkp[:, :, 1::2] << 8)).astype(np.uint16)

    # x^T per core in (t,p) column order: col j <- node c*NPC + (j%128)*BLOCKS
    # + j//128.  40-level quantization with a per-node f16 scale: levels
    # (k - 19.5) * s, s = absmax/19.5; 3 values per int16 word in base 40.
    # Pad nodes get scale 0 (their aug row is scale * psum = 0), so their
    # nonzero dequant values (min |q-19.5| = 0.5) are harmless.
    rs = np.abs(x).max(axis=1) / 19.5
    rs16 = np.maximum(rs, 1e-8).astype(np.float16)
    q_all = np.clip(np.round(x / rs16.astype(np.float32)[:, None] + 19.5),
                    0, 39).astype(np.int64)
    tt = np.arange(NPAD) // 128
    pp = np.arange(NPAD) % 128
    l_of_col = pp * BLOCKS + tt
    ok = l_of_col < NPC
    XQW = (NPAD + 2) // 3
    x6 = np.zeros((NCORES, F, XQW), np.uint16)
    scl = np.zeros((NCORES, 128, BLOCKS), np.float16)
    for c in range(NCORES):
        cols = np.where(ok, c * NPC + np.minimum(l_of_col, NPC - 1), 0)
        q = np.zeros((F, 3 * XQW), np.int64)
        q[:, :NPAD] = q_all[cols].T                           # pads: q=0
        x6[c] = (q[:, 0::3] + 40 * q[:, 1::3]
                 + 1600 * q[:, 2::3]).astype(np.uint16)
        # scale for node (lane p, block t); 0 for pad nodes
        l_g = np.arange(128)[:, None] * BLOCKS + np.arange(BLOCKS)[None, :]
        okg = l_g < NPC
        scl[c] = np.where(
            okg, rs16[c * NPC + np.minimum(l_g, NPC - 1)], np.float16(0.0))

    # graph id per (lane p, block t) node; 200 for pad; u8 packed in pairs
    l_g = np.arange(128)[:, None] * BLOCKS + np.arange(BLOCKS)[None, :]
    okg = l_g < NPC
    GIDW = (BLOCKS + 1) // 2
    gid2 = np.zeros((NCORES, 128, GIDW), np.uint16)
    for c in range(NCORES):
        g8 = np.where(okg, batch[c * NPC + np.minimum(l_g, NPC - 1)],
                      200).astype(np.uint16)
        gp = np.zeros((128, 2 * GIDW), np.uint16)
        gp[:, :BLOCKS] = g8
        gid2[c] = gp[:, 0::2] | (gp[:, 1::2] << 8)

    cnt = np.bincount(batch, minlength=G).astype(np.float32)
    inv_cnt = (1.0 / np.maximum(cnt, 1.0)).astype(np.float32)

    b1 = np.asarray(inputs["b1"], np.float32)
    b2 = np.asarray(inputs["b2"], np.float32)
    l1b = np.asarray(inputs["lin1_b"], np.float32)
    l2b = np.asarray(inputs["lin2_b"], np.float32)
    meta = dict(cfg, NT=NT, U=min(cfg["U"], NT), TBS=tuple(int(t) for t in TBS),
                OFT=tuple(int(t) for t in oft),
                bias1=bool(np.any(b1 != 0)), bias2=bool(np.any(b2 != 0)),
                lbias1=bool(np.any(l1b != 0)), lbias2=bool(np.any(l2b != 0)))
    o = _offsets(meta)

    # weights: each core carries 1/8 of [w1aug | w2aug] (AllGathered on dev)
    w16 = np.concatenate([w1aug, w2aug], 1).astype(np.float16)  # [128, 576]
    # shared f32 tail: lin1W (cols 0:16), inv_cnt (col 16, parts 0:64),
    # lin2W (cols 17:27, parts 0:16); sharded 4 cols per core
    t32 = np.zeros((128, 8 * o["TLW"]), np.float32)
    t32[:, 0:16] = np.asarray(inputs["lin1_W"], np.float32)
    t32[0:G, 16] = inv_cnt
    t32[0:16, 17:17 + cfg["NCLS"]] = np.asarray(inputs["lin2_W"], np.float32)

    in_maps = []
    for c in range(NCORES):
        md = np.zeros((128, o["W_ALL"]), np.int16)
        md[:, o["OX6"]:o["OX6"] + o["X6W"]] = x6[c].view(np.int16)
        md[:, o["OSC"]:o["OSC"] + BLOCKS] = scl[c].view(np.int16)
        md[:, o["OSIDX"]:o["OSIDX"] + NT] = sidxT[c]
        md[:, o["OPK2"]:o["OPK2"] + o["PKW"]] = pk2[c].view(np.int16)
        md[:, o["OGID2"]:o["OGID2"] + o["GIDW"]] = gid2[c].view(np.int16)
        md[:, o["OWSH"]:o["OWSH"] + o["WSH"]] = \
            w16[:, c * o["WSH"]:(c + 1) * o["WSH"]].view(np.int16)
        md[:, o["OTL"]:o["OTL"] + 2 * o["TLW"]] = \
            np.ascontiguousarray(
                t32[:, c * o["TLW"]:(c + 1) * o["TLW"]]).view(np.int16)
        md[:, o["OCN"]:o["OCN"] + 2] = \
            np.full((128, 1), c * NPAD, np.float32).view(np.int16)
        m = dict(md=md)
        if meta["bias1"]:
            m["b1rep"] = np.broadcast_to(b1.astype(np.float32), (128, F)).copy()
        if meta["bias2"]:
            m["b2rep"] = np.broadcast_to(b2.astype(np.float32), (128, F)).copy()
        if meta["lbias1"]:
            m["l1brep"] = np.broadcast_to(l1b, (G, l1b.shape[0])).copy()
        if meta["lbias2"]:
            m["l2brep"] = np.broadcast_to(l2b, (G, l2b.shape[0])).copy()
        in_maps.append(m)
    return meta, in_maps


def build_nc(meta):
    F, H, C, G, NCLS = meta["F"], meta["H"], meta["C"], meta["G"], meta["NCLS"]
    BLOCKS, NPAD, TBLROWS = meta["BLOCKS"], meta["NPAD"], meta["TBLROWS"]
    NPC = meta["NPC"]
    NT, U, AUGW, TBS = meta["NT"], meta["U"], meta["AUGW"], meta["TBS"]
    REPW = 2 * F + H  # matmul rhs width: [hE*exE | hO*exO | ex]
    o = _offsets(meta)
    W_ALL = o["W_ALL"]
    NT2 = o["PKW"] * 2
    # tile -> (block, k-within-block)
    tilemap = [(b, k) for b in range(BLOCKS) for k in range(TBS[b])]
    OFT = meta["OFT"]

    # 2 SWDGE queues: the h-gather and a-gather generate their descriptors
    # on separate queues so the Q7 descriptor generation (the edge-phase
    # bottleneck) for the two streams can overlap.
    nc = bacc.Bacc("TRN2", target_bir_lowering=False, debug=False,
                   num_devices=NCORES, num_swdge_queues=2)

    # --- I/O ---
    d_m = nc.dram_tensor("md", [128, W_ALL], I16, kind="ExternalInput")
    d_bias1 = (nc.dram_tensor("b1rep", [128, F], F32, kind="ExternalInput")
               if meta["bias1"] else None)
    d_bias2 = (nc.dram_tensor("b2rep", [128, F], F32, kind="ExternalInput")
               if meta["bias2"] else None)
    d_l1b = (nc.dram_tensor("l1brep", [G, C], F32, kind="ExternalInput")
             if meta["lbias1"] else None)
    d_l2b = (nc.dram_tensor("l2brep", [G, NCLS], F32, kind="ExternalInput")
             if meta["lbias2"] else None)
    d_out = nc.dram_tensor("out", [2 * G, NCLS], F32, kind="ExternalOutput")

    # --- internal DRAM (collectives + reformatted gather tables) ---
    aug_loc = [nc.dram_tensor(f"aug_loc{i}", [NPAD, AUGW], F16) for i in (1, 2)]
    table = [nc.dram_tensor(f"table{i}", [TBLROWS, AUGW], F16, addr_space="Shared")
             for i in (1, 2)]
    # hp: pair rows [h_e|h_o|as_e|as_o|pad] (768B); ap: pair rows with the
    # a slices at cols 48:64 (even) / 112:128 (odd) (256B)
    hp_tbl = [nc.dram_tensor(f"hp{i}", [TBLROWS // 2, HPW], F16) for i in (1, 2)]
    ap_tbl = [nc.dram_tensor(f"ap{i}", [TBLROWS // 2, 128], F16) for i in (1, 2)]
    wpart = nc.dram_tensor("wpart", [128, o["WSH"]], F16)
    wfull = nc.dram_tensor("wfull", [NCORES * 128, o["WSH"]], F16,
                           addr_space="Shared")
    tpart = nc.dram_tensor("tpart", [128, o["TLW"]], F32)
    tfull = nc.dram_tensor("tfull", [NCORES * 128, o["TLW"]], F32,
                           addr_space="Shared")
    pool_part = nc.dram_tensor("pool_part", [G, F], F32)
    pool_full = nc.dram_tensor("pool_full", [G, F], F32, addr_space="Shared")
    RG = [list(range(NCORES))]

    from contextlib import ExitStack
    with tile.TileContext(nc) as tc, ExitStack() as ctx:
        cpool = ctx.enter_context(tc.tile_pool(name="consts", bufs=1))
        # unpack scratch lives only until the streams are derived; its pool
        # is released before the big edge-phase pools are allocated below
        ustack = ExitStack()
        upool = ustack.enter_context(tc.tile_pool(name="unpk", bufs=1))
        psp = ctx.enter_context(tc.tile_pool(name="ps", bufs=3, space="PSUM"))
        pst = ctx.enter_context(tc.tile_pool(name="pst", bufs=2, space="PSUM"))
        psa = ctx.enter_context(tc.tile_pool(name="psa", bufs=2, space="PSUM"))
        psg = ctx.enter_context(tc.tile_pool(name="psg", bufs=1, space="PSUM"))

        # ================= f32 integer helpers (no DVE int mod) ============
        def f_floordiv(out, in_, tmp, tmp2, m):
            """out = floor(in_ / m) for 0 <= in_ < 2^23, m a power of two.
            round-to-int via +2^23 bias, then -1 + (v >= rnd) fixes any tie
            direction.  out/tmp/tmp2 must be distinct f32 APs; out may NOT
            alias in_."""
            nc.vector.tensor_scalar(out=tmp, in0=in_, scalar1=1.0 / m,
                                    scalar2=None, op0=AX.mult)
            nc.vector.tensor_scalar(out=tmp2, in0=tmp, scalar1=C23,
                                    scalar2=C23, op0=AX.add, op1=AX.subtract)
            nc.vector.tensor_tensor(out=tmp, in0=tmp, in1=tmp2, op=AX.subtract)
            nc.vector.tensor_scalar(out=tmp, in0=tmp, scalar1=0.0,
                                    scalar2=None, op0=AX.is_ge)
            nc.vector.scalar_tensor_tensor(out=out, in0=tmp2, scalar=-1.0,
                                           op0=AX.add, in1=tmp, op1=AX.add)

        # ---- load + unpack the base-40 x^T blob (chunked scratch) ----
        # word w = q0 + 40*q1 + 1600*q2, q in [0,40); xt value = q - 19.5.
        # floor(v/m) for arbitrary m: rnd via +2^23 bias on v*(1/m) (approx),
        # then the EXACT integer d = v - m*rnd picks the fix direction.
        XQW = o["X6W"]
        NPADX = 3 * XQW
        xt_sb = cpool.tile([F, NPADX], F16, tag="xt")
        scale_sb = cpool.tile([128, BLOCKS], F16, tag="scl")
        nc.sync.dma_start(out=scale_sb[:],
                          in_=d_m[:, o["OSC"]:o["OSC"] + BLOCKS].bitcast(F16))
        sclf_sb = cpool.tile([128, BLOCKS], F32, tag="sclf")
        nc.vector.tensor_copy(out=sclf_sb[:], in_=scale_sb[:])

        def f_floordiv_any(out, in_, tmp, tmp2, m):
            """out = floor(in_ / m) for 0 <= in_ < 2^16, any integer m."""
            nc.vector.tensor_scalar(out=tmp, in0=in_, scalar1=1.0 / m,
                                    scalar2=None, op0=AX.mult)
            nc.vector.tensor_scalar(out=tmp2, in0=tmp, scalar1=C23,
                                    scalar2=C23, op0=AX.add, op1=AX.subtract)
            nc.vector.scalar_tensor_tensor(out=tmp, in0=tmp2,
                                           scalar=-float(m), op0=AX.mult,
                                           in1=in_, op1=AX.add)
            nc.vector.tensor_scalar(out=tmp, in0=tmp, scalar1=0.0,
                                    scalar2=None, op0=AX.is_ge)
            nc.vector.scalar_tensor_tensor(out=out, in0=tmp2, scalar=-1.0,
                                           op0=AX.add, in1=tmp, op1=AX.add)

        NCHUNK = 4
        NGC = (XQW + NCHUNK - 1) // NCHUNK   # words per chunk
        xqc = upool.tile([128, NGC], I16, tag="xqc")
        wfu = upool.tile([128, NGC], F32, tag="wfu")
        tA = upool.tile([128, NGC], F32, tag="tA")
        tB = upool.tile([128, NGC], F32, tag="tB")
        tC = upool.tile([128, NGC], F32, tag="tC")
        tD = upool.tile([128, NGC], F32, tag="tD")

        for ch in range(NCHUNK):
            g0 = ch * NGC
            gn = min(NGC, XQW - g0)
            if gn <= 0:
                break
            nc.sync.dma_start(
                out=xqc[:, :gn],
                in_=d_m[:, o["OX6"] + g0:o["OX6"] + g0 + gn])
            w = wfu[:, :gn]
            nc.vector.tensor_copy(out=w, in_=xqc[:, :gn])
            nc.vector.tensor_scalar(out=tA[:, :gn], in0=w, scalar1=0.0,
                                    scalar2=None, op0=AX.is_lt)
            nc.vector.scalar_tensor_tensor(out=w, in0=tA[:, :gn],
                                           scalar=65536.0, op0=AX.mult,
                                           in1=w, op1=AX.add)
            x3 = xt_sb[:, 3 * g0:3 * (g0 + gn)].rearrange(
                "p (g three) -> p g three", three=3)
            wv = w.rearrange("p (g one) -> p g one", one=1)
            vA = tA[:, :gn].rearrange("p (g one) -> p g one", one=1)
            vB = tB[:, :gn].rearrange("p (g one) -> p g one", one=1)
            vC = tC[:, :gn].rearrange("p (g one) -> p g one", one=1)
            vD = tD[:, :gn].rearrange("p (g one) -> p g one", one=1)
            # q2 = floor(w/1600); r = w - 1600*q2
            f_floordiv_any(vA, wv, vC, vD, 1600)
            nc.vector.tensor_scalar(out=x3[:, :, 2:3], in0=vA, scalar1=19.5,
                                    scalar2=None, op0=AX.subtract)
            nc.vector.scalar_tensor_tensor(out=vB, in0=vA, scalar=-1600.0,
                                           op0=AX.mult, in1=wv, op1=AX.add)
            # q1 = floor(r/40); q0 = r - 40*q1
            f_floordiv_any(vA, vB, vC, vD, 40)
            nc.vector.tensor_scalar(out=x3[:, :, 1:2], in0=vA, scalar1=19.5,
                                    scalar2=None, op0=AX.subtract)
            nc.vector.scalar_tensor_tensor(out=vC, in0=vA, scalar=-40.0,
                                           op0=AX.mult, in1=vB, op1=AX.add)
            nc.vector.tensor_scalar(out=x3[:, :, 0:1], in0=vC, scalar1=19.5,
                                    scalar2=None, op0=AX.subtract)

        # ---- edge-stream unpack ----
        # src: int16 sidx -> f32; sign bit = parity, low 15 bits = pair row
        sidx_sb = upool.tile([128, NT], I16, tag="sidx")
        nc.sync.dma_start(out=sidx_sb[:],
                          in_=d_m[:, o["OSIDX"]:o["OSIDX"] + NT])
        sv = upool.tile([128, NT], F32, tag="sv")
        nc.vector.tensor_copy(out=sv[:], in_=sidx_sb[:])
        psrcf = upool.tile([128, NT], F32, tag="psrcf")
        nc.vector.tensor_scalar(out=psrcf[:], in0=sv[:], scalar1=0.0,
                                scalar2=None, op0=AX.is_lt)
        psrc_sb = cpool.tile([128, NT], F16, tag="psrc")
        qsrc_sb = cpool.tile([128, NT], F16, tag="qsrc")
        nc.vector.tensor_copy(out=psrc_sb[:], in_=psrcf[:])
        nc.vector.tensor_scalar(out=qsrc_sb[:], in0=psrc_sb[:], scalar1=-1.0,
                                scalar2=1.0, op0=AX.mult, op1=AX.add)
        # spos = sv + 32768*psrc (f32) = src pair row in [0, 32768)
        spos = upool.tile([128, NT], F32, tag="spos")
        nc.vector.scalar_tensor_tensor(out=spos[:], in0=psrcf[:],
                                       scalar=32768.0, op0=AX.mult,
                                       in1=sv[:], op1=AX.add)
        ph16s = upool.tile([128, NT], I16, tag="ph16s")
        nc.vector.tensor_copy(out=ph16s[:], in_=spos[:])

        # dst payload: u8 pairs -> pkf [128, NT2] f16, then lane/parity
        pk2_sb = upool.tile([128, o["PKW"]], I16, tag="pk2")
        nc.sync.dma_start(out=pk2_sb[:],
                          in_=d_m[:, o["OPK2"]:o["OPK2"] + o["PKW"]])
        pv = upool.tile([128, o["PKW"]], F32, tag="pv")
        nc.vector.tensor_copy(out=pv[:], in_=pk2_sb[:])
        pneg = upool.tile([128, o["PKW"]], F32, tag="pneg")
        nc.vector.tensor_scalar(out=pneg[:], in0=pv[:], scalar1=0.0,
                                scalar2=None, op0=AX.is_lt)
        nc.vector.scalar_tensor_tensor(out=pv[:], in0=pneg[:], scalar=65536.0,
                                       op0=AX.mult, in1=pv[:], op1=AX.add)
        phi = upool.tile([128, o["PKW"]], F32, tag="phi")
        pt2 = upool.tile([128, o["PKW"]], F32, tag="pt2")
        f_floordiv(phi[:].rearrange("p (u one) -> p u one", one=1),
                   pv[:].rearrange("p (u one) -> p u one", one=1),
                   pneg[:].rearrange("p (u one) -> p u one", one=1),
                   pt2[:].rearrange("p (u one) -> p u one", one=1),
                   256)
        pkf = cpool.tile([128, NT2], F16, tag="pkf")
        pk2v = pkf[:].rearrange("p (u two) -> p u two", two=2)
        nc.vector.scalar_tensor_tensor(
            out=pk2v[:, :, 0:1],
            in0=phi[:].rearrange("p (u one) -> p u one", one=1),
            scalar=-256.0, op0=AX.mult,
            in1=pv[:].rearrange("p (u one) -> p u one", one=1), op1=AX.add)
        nc.vector.tensor_copy(
            out=pk2v[:, :, 1:2],
            in_=phi[:].rearrange("p (u one) -> p u one", one=1))
        pdst_sb = cpool.tile([128, NT], F16, tag="pdst")
        dstl_sb = cpool.tile([128, NT], F16, tag="dstl")
        nc.vector.tensor_scalar(out=pdst_sb[:], in0=pkf[:, 0:NT],
                                scalar1=128.0, scalar2=None, op0=AX.is_ge)
        nc.vector.scalar_tensor_tensor(out=dstl_sb[:], in0=pdst_sb[:],
                                       scalar=-128.0, op0=AX.mult,
                                       in1=pkf[:, 0:NT], op1=AX.add)

        # gid: u8 pairs -> gid_sb [128, BLOCKS] f16
        gid2_sb = upool.tile([128, o["GIDW"]], I16, tag="gid2")
        nc.sync.dma_start(out=gid2_sb[:],
                          in_=d_m[:, o["OGID2"]:o["OGID2"] + o["GIDW"]])
        gv = upool.tile([128, o["GIDW"]], F32, tag="gv")
        nc.vector.tensor_copy(out=gv[:], in_=gid2_sb[:])
        gneg = upool.tile([128, o["GIDW"]], F32, tag="gneg")
        nc.vector.tensor_scalar(out=gneg[:], in0=gv[:], scalar1=0.0,
                                scalar2=None, op0=AX.is_lt)
        nc.vector.scalar_tensor_tensor(out=gv[:], in0=gneg[:], scalar=65536.0,
                                       op0=AX.mult, in1=gv[:], op1=AX.add)
        ghi = upool.tile([128, o["GIDW"]], F32, tag="ghi")
        gt2 = upool.tile([128, o["GIDW"]], F32, tag="gt2")
        f_floordiv(ghi[:].rearrange("p (u one) -> p u one", one=1),
                   gv[:].rearrange("p (u one) -> p u one", one=1),
                   gneg[:].rearrange("p (u one) -> p u one", one=1),
                   gt2[:].rearrange("p (u one) -> p u one", one=1),
                   256)
        gid_sb = cpool.tile([128, 2 * o["GIDW"]], F16, tag="gid")
        gid2v = gid_sb[:].rearrange("p (u two) -> p u two", two=2)
        nc.vector.scalar_tensor_tensor(
            out=gid2v[:, :, 0:1],
            in0=ghi[:].rearrange("p (u one) -> p u one", one=1),
            scalar=-256.0, op0=AX.mult,
            in1=gv[:].rearrange("p (u one) -> p u one", one=1), op1=AX.add)
        nc.vector.tensor_copy(
            out=gid2v[:, :, 1:2],
            in_=ghi[:].rearrange("p (u one) -> p u one", one=1))

        # src gather stream: wrap-shuffle ph16s into dma_gather's layout
        # ([16 partitions, NT*8]; element (r, u*8+q) = pair row of edge slot
        # (partition q*16+r, tile u)) while the unpack scratch is still live
        idxr = cpool.tile([128, NT * 16], I16, tag="idxr")
        wraps = idxr[0:16, 0:NT * 8].rearrange("r (u q) -> r u q", q=8)
        for q in range(8):
            nc.sync.dma_start(
                out=wraps[:, :, q:q + 1],
                in_=ph16s[q * 16:(q + 1) * 16, :].rearrange(
                    "p (u one) -> p u one", one=1))
        ustack.close()  # release the unpack scratch pool

        # edge-phase pools, allocated in the space the unpack scratch used
        gpool = ctx.enter_context(tc.tile_pool(name="gath", bufs=2))
        hpool = ctx.enter_context(tc.tile_pool(name="hsex", bufs=2))
        opool = ctx.enter_context(tc.tile_pool(name="oneh", bufs=2))
        zpool = ctx.enter_context(tc.tile_pool(name="zl", bufs=3))
        apool = ctx.enter_context(tc.tile_pool(name="adL", bufs=2))
        epool = ctx.enter_context(tc.tile_pool(name="epi", bufs=3))
        augp = ctx.enter_context(tc.tile_pool(name="augsb", bufs=2))

        # ---- AllGather the weight + tail shards ----
        nc.sync.dma_start(out=wpart[:, :],
                          in_=d_m[:, o["OWSH"]:o["OWSH"] + o["WSH"]].bitcast(F16))
        nc.gpsimd.collective_compute(
            "AllGather", AX.bypass, replica_groups=RG,
            ins=[wpart[:, :].opt()], outs=[wfull[:, :].opt()])
        w_sb = cpool.tile([128, 2 * AUGW], F16, tag="wsb")
        for c in range(NCORES):
            nc.sync.dma_start(out=w_sb[:, c * o["WSH"]:(c + 1) * o["WSH"]],
                              in_=wfull[c * 128:(c + 1) * 128, :])
        nc.sync.dma_start(
            out=tpart[:, :],
            in_=d_m[:, o["OTL"]:o["OTL"] + 2 * o["TLW"]].bitcast(F32))
        nc.gpsimd.collective_compute(
            "AllGather", AX.bypass, replica_groups=RG,
            ins=[tpart[:, :].opt()], outs=[tfull[:, :].opt()])
        t32_sb = cpool.tile([128, 8 * o["TLW"]], F32, tag="t32")
        for c in range(NCORES):
            nc.sync.dma_start(out=t32_sb[:, c * o["TLW"]:(c + 1) * o["TLW"]],
                              in_=tfull[c * 128:(c + 1) * 128, :])
        cn_sb = cpool.tile([128, 1], F32, tag="cn")
        nc.sync.dma_start(out=cn_sb[:],
                          in_=d_m[:, o["OCN"]:o["OCN"] + 2].bitcast(F32))

        bias1_sb = bias2_sb = l1b_sb = l2b_sb = None
        if d_bias1 is not None:
            bias1_sb = cpool.tile([128, F], F32, tag="b1")
            nc.sync.dma_start(out=bias1_sb[:], in_=d_bias1[:, :])
        if d_bias2 is not None:
            bias2_sb = cpool.tile([128, F], F32, tag="b2")
            nc.sync.dma_start(out=bias2_sb[:], in_=d_bias2[:, :])
        if d_l1b is not None:
            l1b_sb = cpool.tile([G, C], F32, tag="l1b")
            nc.sync.dma_start(out=l1b_sb[:], in_=d_l1b[:, :])
        if d_l2b is not None:
            l2b_sb = cpool.tile([G, NCLS], F32, tag="l2b")
            nc.sync.dma_start(out=l2b_sb[:], in_=d_l2b[:, :])

        # ---- device-built constants ----
        iota_sb = cpool.tile([128, U * 128], F16, tag="iota")
        nc.gpsimd.iota(out=iota_sb[:], pattern=[[0, U], [1, 128]], base=0,
                       channel_multiplier=0,
                       allow_small_or_imprecise_dtypes=True)
        # identities via two iotas + is_equal (no negative channel mult)
        idh_sb = cpool.tile([128, 128], F16, tag="idh")
        rowh = cpool.tile([128, 128], F16, tag="rowh")
        nc.gpsimd.iota(out=idh_sb[:], pattern=[[1, 128]], base=0,
                       channel_multiplier=0,
                       allow_small_or_imprecise_dtypes=True)
        nc.gpsimd.iota(out=rowh[:], pattern=[[0, 128]], base=0,
                       channel_multiplier=1,
                       allow_small_or_imprecise_dtypes=True)
        nc.vector.tensor_tensor(out=idh_sb[:], in0=idh_sb[:], in1=rowh[:],
                                op=AX.is_equal)
        idf_sb = cpool.tile([64, 64], F32, tag="idf")
        rowf = cpool.tile([64, 64], F32, tag="rowf")
        nc.gpsimd.iota(out=idf_sb[:], pattern=[[1, 64]], base=0,
                       channel_multiplier=0,
                       allow_small_or_imprecise_dtypes=True)
        nc.gpsimd.iota(out=rowf[:], pattern=[[0, 64]], base=0,
                       channel_multiplier=1,
                       allow_small_or_imprecise_dtypes=True)
        nc.vector.tensor_tensor(out=idf_sb[:], in0=idf_sb[:], in1=rowf[:],
                                op=AX.is_equal)
        # graph one-hot: gone[p, t*G+g] = (gid[p,t] == g)
        gone_sb = cpool.tile([128, BLOCKS * G], F16, tag="gone")
        nc.gpsimd.iota(out=gone_sb[:], pattern=[[0, BLOCKS], [1, G]], base=0,
                       channel_multiplier=0,
                       allow_small_or_imprecise_dtypes=True)
        nc.vector.tensor_tensor(
            out=gone_sb[:].rearrange("p (t g) -> p t g", g=G),
            in0=gone_sb[:].rearrange("p (t g) -> p t g", g=G),
            in1=gid_sb[:, 0:BLOCKS].to_broadcast([128, BLOCKS, G]),
            op=AX.is_equal)

        # ---- derive the dst gather stream on device ----
        # dst pair row = (c*NPAD + dstl*BLOCKS + b - pdst) / 2, computed in
        # f32 (exact), cast to i16 and wrap-shuffled into the dst half of
        # idxr (cols NT*8:NT*16).
        drow = cpool.tile([128, NT], F32, tag="drow")
        for b in range(BLOCKS):
            nc.vector.memset(drow[:, OFT[b]:OFT[b + 1]], float(b))
        nc.vector.scalar_tensor_tensor(out=drow[:], in0=dstl_sb[:],
                                       scalar=float(BLOCKS), op0=AX.mult,
                                       in1=drow[:], op1=AX.add)
        nc.vector.tensor_scalar(out=drow[:], in0=drow[:],
                                scalar1=cn_sb[:, 0:1], scalar2=None,
                                op0=AX.add)
        nc.vector.scalar_tensor_tensor(out=drow[:], in0=pdst_sb[:],
                                       scalar=-1.0, op0=AX.mult,
                                       in1=drow[:], op1=AX.add)
        nc.vector.tensor_scalar(out=drow[:], in0=drow[:], scalar1=0.5,
                                scalar2=float(TBLROWS // 2 - 1), op0=AX.mult,
                                op1=AX.min)
        ph16 = cpool.tile([128, NT], I16, tag="ph16")
        nc.vector.tensor_copy(out=ph16[:], in_=drow[:])
        wrapv = idxr[0:16, NT * 8:NT * 16].rearrange("r (u q) -> r u q", q=8)
        for q in range(8):
            nc.sync.dma_start(
                out=wrapv[:, :, q:q + 1],
                in_=ph16[q * 16:(q + 1) * 16, :].rearrange(
                    "p (u one) -> p u one", one=1))
        # replicate 16 -> 128 partitions (dma_gather wants the stream on
        # every 16-partition group)
        nc.sync.dma_start(out=idxr[16:32, :], in_=idxr[0:16, :])
        nc.sync.dma_start(out=idxr[32:64, :], in_=idxr[0:32, :])
        nc.sync.dma_start(out=idxr[64:128, :], in_=idxr[0:64, :])

        # pad-row mask: msk[p, t] = -3e4 where local node l = p*BLOCKS+t is a
        # pad (l >= NPC), else 0.  Adding it to asrc/adst makes any gather of
        # a pad row produce ex = exp(leaky(-3e4 + .)) = 0.
        lpos_sb = cpool.tile([128, BLOCKS], F32, tag="lpos")
        nc.gpsimd.iota(out=lpos_sb[:], pattern=[[1, BLOCKS]], base=0,
                       channel_multiplier=BLOCKS,
                       allow_small_or_imprecise_dtypes=True)
        msk_sb = cpool.tile([128, BLOCKS], F16, tag="msk")
        nc.vector.tensor_scalar(out=msk_sb[:], in0=lpos_sb[:],
                                scalar1=float(NPC), scalar2=PAD_A,
                                op0=AX.is_ge, op1=AX.mult)

        def mask_pad_aug(aug_sb):
            a3 = aug_sb[:].rearrange("p (t a) -> p t a", a=AUGW)
            nc.vector.tensor_tensor(
                out=a3[:, :, F:F + 2 * H], in0=a3[:, :, F:F + 2 * H],
                in1=msk_sb[:].to_broadcast([128, BLOCKS, 2 * H]), op=AX.add)

        def build_aug_from_xt():
            """aug rows for own nodes from resident x^T (6-bit ints); the
            per-node dequant scale is folded into the psum->sbuf copy."""
            aug_sb = augp.tile([128, BLOCKS * AUGW], F16, tag="augsb")
            for t in range(BLOCKS):
                ps = psa.tile([128, AUGW], F32, tag="psaug")
                nc.tensor.matmul(out=ps[:], lhsT=xt_sb[:, t * 128:(t + 1) * 128],
                                 rhs=w_sb[:, 0:AUGW],
                                 start=True, stop=True)
                nc.vector.tensor_scalar(
                    out=aug_sb[:, t * AUGW:(t + 1) * AUGW], in0=ps[:],
                    scalar1=sclf_sb[:, t:t + 1], scalar2=None, op0=AX.mult)
            return aug_sb

        def publish_table(aug_sb, which):
            dst = aug_loc[which]
            # DRAM rows r = p*BLOCKS + t  <=> view [(p t), f] -> [p, (t f)]
            nc.sync.dma_start(
                out=dst[:, :].rearrange("(p t) f -> p (t f)", t=BLOCKS),
                in_=aug_sb[:])
            nc.gpsimd.collective_compute(
                "AllGather", AX.bypass, replica_groups=RG,
                ins=[dst[:, :].opt()], outs=[table[which][:, :].opt()])
            # reformat into pair-row gather tables (DRAM->DRAM)
            t3 = table[which][:, :].rearrange("(g two) f -> g two f", two=2)
            nc.sync.dma_start(
                out=hp_tbl[which][:, 0:2 * F].rearrange(
                    "g (two f) -> g two f", two=2),
                in_=t3[:, :, 0:F])
            nc.sync.dma_start(
                out=hp_tbl[which][:, 2 * F:2 * F + 2 * H].rearrange(
                    "g (two a) -> g two a", two=2),
                in_=t3[:, :, F:F + H])
            # full 128-col rows (finite pad): cols 48:64 = a_even,
            # cols 112:128 = a_odd; 0:48/64:112 are h-tail junk
            nc.sync.dma_start(
                out=ap_tbl[which][:, :].rearrange("g (two j) -> g two j", two=2),
                in_=t3[:, :, F - 48:F + 2 * H])

        def elu_inplace(v_sb, width, out_tile):
            """out_tile(fp16) = elu(v_sb) = max(v,0) + min(exp(v)-1, 0)."""
            t_sb = epool.tile([128, width], F32, tag="elu_t")
            nc.scalar.activation(out=t_sb[:], in_=v_sb[:],
                                 func=mybir.ActivationFunctionType.Exp)
            nc.vector.tensor_scalar(out=t_sb[:], in0=t_sb[:], scalar1=1.0,
                                    scalar2=0.0, op0=AX.subtract, op1=AX.min)
            nc.vector.scalar_tensor_tensor(out=out_tile[:], in0=v_sb[:],
                                           scalar=0.0, op0=AX.max,
                                           in1=t_sb[:], op1=AX.add)

        def edge_phase(layer, aug_sb):
            """layer 0: consumes table[0], produces aug tile for table[1].
               layer 1: consumes table[1], accumulates pool psum.  aug_sb is
               the CURRENT layer's local aug tile (for the dense self-loop
               term).  Returns next aug tile (layer 0) or pool psum."""
            bias_sb = (bias1_sb, bias2_sb)[layer]
            if layer == 0:
                out_aug = augp.tile([128, BLOCKS * AUGW], F16, tag="augsb")
            else:
                pool_ps = psg.tile([G, F], F32, tag="poolps")

            hp, ap = hp_tbl[layer], ap_tbl[layer]
            nbatch = (NT + U - 1) // U
            ps_cur = None
            for bi in range(nbatch):
                u0 = bi * U
                ub = min(U, NT - u0)
                # bulk gathers: [h|asrc] pair-rows by src//2 (768B) and a
                # pair-rows by dst//2 (256B) — 2 descriptors per edge
                ghp = gpool.tile([128, U * HPW], F16, tag="g")
                nc.gpsimd.dma_gather(
                    out_ap=ghp[:, :ub * HPW].rearrange(
                        "p (u f) -> p u f", f=HPW),
                    in_ap=hp[:, :], idxs_ap=idxr[:, u0 * 8:(u0 + ub) * 8],
                    num_idxs=ub * 128, num_idxs_reg=ub * 128, elem_size=HPW,
                    single_packet=False)
                gap = apool.tile([128, U * 128], F16, tag="gap")
                nc.gpsimd.dma_gather(
                    out_ap=gap[:, :ub * 128].rearrange(
                        "p (u f) -> p u f", f=128),
                    in_ap=ap[:, :],
                    idxs_ap=idxr[:, NT * 8 + u0 * 8:NT * 8 + (u0 + ub) * 8],
                    num_idxs=ub * 128, num_idxs_reg=ub * 128,
                    elem_size=128, single_packet=False, queue_num=1)
                g3 = ghp[:, :ub * HPW].rearrange("p (u f) -> p u f", f=HPW)
                ga = gap[:, :ub * 128].rearrange("p (u f) -> p u f", f=128)

                # z = asrc[src] + adst[dst] with parity selection:
                #   asrc = ae + psrc*(ao-ae); adst = be + pdst*(bo-be)
                zl = zpool.tile([128, U * H], F16, tag="zl")
                tsel = zpool.tile([128, U * H], F16, tag="tsel")
                psB = psrc_sb[:, u0:u0 + ub].to_broadcast([128, ub, H])
                pdB = pdst_sb[:, u0:u0 + ub].to_broadcast([128, ub, H])
                t3 = tsel[:, :ub * H].rearrange("p (u h) -> p u h", h=H)
                z3 = zl[:, :ub * H].rearrange("p (u h) -> p u h", h=H)
                nc.vector.tensor_tensor(out=t3, in0=g3[:, :, 2 * F + H:2 * F + 2 * H],
                                        in1=g3[:, :, 2 * F:2 * F + H],
                                        op=AX.subtract)
                nc.vector.tensor_tensor(out=t3, in0=t3, in1=psB, op=AX.mult)
                nc.vector.tensor_tensor(out=z3, in0=t3,
                                        in1=g3[:, :, 2 * F:2 * F + H], op=AX.add)
                nc.vector.tensor_tensor(out=t3, in0=ga[:, :, 120:128],
                                        in1=ga[:, :, 56:64], op=AX.subtract)
                nc.vector.tensor_tensor(out=t3, in0=t3, in1=pdB, op=AX.mult)
                nc.vector.tensor_tensor(out=z3, in0=z3, in1=t3, op=AX.add)
                nc.vector.tensor_tensor(out=z3, in0=z3,
                                        in1=ga[:, :, 56:64], op=AX.add)
                zv = zl[:, :ub * H]
                nc.vector.scalar_tensor_tensor(
                    out=zv, in0=zv, scalar=0.2, op0=AX.mult, in1=zv, op1=AX.max)

                he = hpool.tile([128, U * REPW], F16, tag="he")
                he3 = he[:, :ub * REPW].rearrange("p (u f) -> p u f", f=REPW)
                nc.scalar.activation(
                    out=he3[:, :, 2 * F:2 * F + H],
                    in_=zl[:, :ub * H].rearrange("p (u h) -> p u h", h=H),
                    func=mybir.ActivationFunctionType.Exp)
                # parity-masked ex, folded into the h scaling: the even half is
                # scaled by ex*(1-psrc), the odd half by ex*psrc, so the wrong
                # parity contributes zero and the psum halves sum to the answer
                exE = zpool.tile([128, U * H], F16, tag="exE")
                exO = zpool.tile([128, U * H], F16, tag="exO")
                eE3 = exE[:, :ub * H].rearrange("p (u h) -> p u h", h=H)
                eO3 = exO[:, :ub * H].rearrange("p (u h) -> p u h", h=H)
                nc.vector.tensor_tensor(
                    out=eE3, in0=he3[:, :, 2 * F:2 * F + H],
                    in1=qsrc_sb[:, u0:u0 + ub].to_broadcast([128, ub, H]),
                    op=AX.mult)
                nc.vector.tensor_tensor(
                    out=eO3, in0=he3[:, :, 2 * F:2 * F + H], in1=psB,
                    op=AX.mult)
                nc.vector.tensor_tensor(
                    out=he3[:, :, 0:F].rearrange("p u (h c) -> p u h c", c=C),
                    in0=g3[:, :, 0:F].rearrange("p u (h c) -> p u h c", c=C),
                    in1=eE3.to_broadcast([128, ub, H, C]), op=AX.mult)
                nc.vector.tensor_tensor(
                    out=he3[:, :, F:2 * F].rearrange("p u (h c) -> p u h c", c=C),
                    in0=g3[:, :, F:2 * F].rearrange("p u (h c) -> p u h c", c=C),
                    in1=eO3.to_broadcast([128, ub, H, C]), op=AX.mult)

                oh = opool.tile([128, U * 128], F16, tag="oh")
                nc.vector.tensor_tensor(
                    out=oh[:, :ub * 128].rearrange("p (u j) -> p u j", j=128),
                    in0=iota_sb[:, :ub * 128].rearrange("p (u j) -> p u j", j=128),
                    in1=dstl_sb[:, u0:u0 + ub].to_broadcast([128, ub, 128]),
                    op=AX.is_equal)

                for u in range(ub):
                    t = u0 + u
                    b, k = tilemap[t]
                    if k == 0:
                        ps_cur = psp.tile([128, REPW], F32, tag="psblk")
                    nc.tensor.matmul(
                        out=ps_cur[:], lhsT=oh[:, u * 128:(u + 1) * 128],
                        rhs=he[:, u * REPW:(u + 1) * REPW],
                        start=(k == 0), stop=(k == TBS[b] - 1))
                    if k == TBS[b] - 1:
                        # ---- block epilogue ----
                        # dense self-loop term from the local aug rows:
                        # ex_self = exp(leaky(asrc+adst)), s += ex_self,
                        # num += h_local * ex_self
                        zs = epool.tile([128, H], F16, tag="zs")
                        nc.vector.tensor_tensor(
                            out=zs[:], in0=aug_sb[:, b * AUGW + F:b * AUGW + F + H],
                            in1=aug_sb[:, b * AUGW + F + H:b * AUGW + F + 2 * H],
                            op=AX.add)
                        nc.vector.scalar_tensor_tensor(
                            out=zs[:], in0=zs[:], scalar=0.2, op0=AX.mult,
                            in1=zs[:], op1=AX.max)
                        exs = epool.tile([128, H], F32, tag="exs")
                        nc.scalar.activation(out=exs[:], in_=zs[:],
                                             func=mybir.ActivationFunctionType.Exp)
                        s_sb = epool.tile([128, H], F32, tag="s")
                        nc.vector.tensor_tensor(out=s_sb[:], in0=exs[:],
                                                in1=ps_cur[:, 2 * F:2 * F + H],
                                                op=AX.add)
                        nc.vector.tensor_scalar(out=s_sb[:], in0=s_sb[:],
                                                scalar1=1e-30, scalar2=None,
                                                op0=AX.max)
                        r_sb = epool.tile([128, H], F32, tag="r")
                        nc.vector.reciprocal(out=r_sb[:], in_=s_sb[:])
                        hs_sb = epool.tile([128, F], F32, tag="hs")
                        nc.vector.tensor_tensor(
                            out=hs_sb[:].rearrange("p (h c) -> p h c", c=C),
                            in0=aug_sb[:, b * AUGW:b * AUGW + F].rearrange(
                                "p (h c) -> p h c", c=C),
                            in1=exs[:].to_broadcast([128, H, C]), op=AX.mult)
                        hc_sb = epool.tile([128, F], F32, tag="hc")
                        nc.vector.tensor_tensor(out=hc_sb[:], in0=hs_sb[:],
                                                in1=ps_cur[:, 0:F], op=AX.add)
                        nc.vector.tensor_tensor(out=hc_sb[:], in0=hc_sb[:],
                                                in1=ps_cur[:, F:2 * F], op=AX.add)
                        v_sb = epool.tile([128, F], F32, tag="v")
                        nc.vector.tensor_tensor(
                            out=v_sb[:].rearrange("p (h c) -> p h c", c=C),
                            in0=hc_sb[:].rearrange("p (h c) -> p h c", c=C),
                            in1=r_sb[:].to_broadcast([128, H, C]), op=AX.mult)
                        if bias_sb is not None:
                            nc.vector.tensor_tensor(out=v_sb[:], in0=v_sb[:],
                                                    in1=bias_sb[:], op=AX.add)
                        eo = epool.tile([128, F], F16, tag="eo")
                        elu_inplace(v_sb, F, eo)
                        if layer == 0:
                            trp = pst.tile([128, 128], F16, tag="trps")
                            nc.tensor.transpose(out=trp[:], in_=eo[:],
                                                identity=idh_sb[:])
                            trs = epool.tile([128, 128], F16, tag="trsb")
                            nc.vector.tensor_copy(out=trs[:], in_=trp[:])
                            ap2 = psa.tile([128, AUGW], F32, tag="psaug")
                            nc.tensor.matmul(out=ap2[:], lhsT=trs[:],
                                             rhs=w_sb[:, AUGW:2 * AUGW],
                                             start=True, stop=True)
                            nc.vector.tensor_copy(
                                out=out_aug[:, b * AUGW:(b + 1) * AUGW],
                                in_=ap2[:])
                        else:
                            nc.tensor.matmul(
                                out=pool_ps[:],
                                lhsT=gone_sb[:, b * G:(b + 1) * G],
                                rhs=eo[:], start=(b == 0), stop=(b == BLOCKS - 1))
            return out_aug if layer == 0 else pool_ps

        # ---------------- pipeline ----------------
        aug1_sb = build_aug_from_xt()
        mask_pad_aug(aug1_sb)
        publish_table(aug1_sb, 0)
        aug2_sb = edge_phase(0, aug1_sb)
        mask_pad_aug(aug2_sb)
        publish_table(aug2_sb, 1)
        pool_ps = edge_phase(1, aug2_sb)

        # pooling allreduce
        psum_sb = epool.tile([G, F], F32, tag="poolsb")
        nc.vector.tensor_copy(out=psum_sb[:], in_=pool_ps[:])
        nc.sync.dma_start(out=pool_part[:, :], in_=psum_sb[:])
        nc.gpsimd.collective_compute(
            "AllReduce", AX.add, replica_groups=RG,
            ins=[pool_part[:, :].opt()], outs=[pool_full[:, :].opt()])
        hg_sb = epool.tile([G, F], F32, tag="hg")
        nc.sync.dma_start(out=hg_sb[:], in_=pool_full[:, :])
        nc.vector.tensor_scalar(out=hg_sb[:], in0=hg_sb[:],
                                scalar1=t32_sb[0:G, 16:17], scalar2=None,
                                op0=AX.mult)

        # MLP: z1 = elu(hg @ lin1W + b); logits = z1 @ lin2W + b
        hgT_ps = pst.tile([F, G], F32, tag="trps")
        nc.tensor.transpose(out=hgT_ps[:], in_=hg_sb[:], identity=idf_sb[:G, :G])
        hgT_sb = epool.tile([F, G], F32, tag="hgTs")
        nc.vector.tensor_copy(out=hgT_sb[:], in_=hgT_ps[:])
        z1_ps = psa.tile([G, C], F32, tag="psaug")
        nc.tensor.matmul(out=z1_ps[:], lhsT=hgT_sb[:], rhs=t32_sb[:, 0:16],
                         start=True, stop=True)
        z1_sb = epool.tile([G, C], F32, tag="z1s")
        if l1b_sb is not None:
            nc.vector.tensor_tensor(out=z1_sb[:], in0=z1_ps[:], in1=l1b_sb[:],
                                    op=AX.add)
        else:
            nc.vector.tensor_copy(out=z1_sb[:], in_=z1_ps[:])
        z1e_sb = epool.tile([G, C], F32, tag="z1e")
        t1 = epool.tile([G, C], F32, tag="t1")
        nc.scalar.activation(out=t1[:], in_=z1_sb[:],
                             func=mybir.ActivationFunctionType.Exp)
        nc.vector.tensor_scalar(out=t1[:], in0=t1[:], scalar1=1.0, scalar2=0.0,
                                op0=AX.subtract, op1=AX.min)
        nc.vector.scalar_tensor_tensor(out=z1e_sb[:], in0=z1_sb[:], scalar=0.0,
                                       op0=AX.max, in1=t1[:], op1=AX.add)
        z1T_ps = pst.tile([C, G], F32, tag="trps")
        nc.tensor.transpose(out=z1T_ps[:], in_=z1e_sb[:], identity=idf_sb[:G, :G])
        z1T_sb = epool.tile([C, G], F32, tag="z1Ts")
        nc.vector.tensor_copy(out=z1T_sb[:], in_=z1T_ps[:])
        lg_ps = psa.tile([G, NCLS], F32, tag="psaug")
        nc.tensor.matmul(out=lg_ps[:], lhsT=z1T_sb[:],
                         rhs=t32_sb[0:16, 17:17 + NCLS],
                         start=True, stop=True)
        lg_sb = epool.tile([G, NCLS], F32, tag="lgs")
        if l2b_sb is not None:
            nc.vector.tensor_tensor(out=lg_sb[:], in0=lg_ps[:], in1=l2b_sb[:],
                                    op=AX.add)
        else:
            nc.vector.tensor_copy(out=lg_sb[:], in_=lg_ps[:])

        # log_softmax
        m_sb = epool.tile([G, 1], F32, tag="m")
        nc.vector.tensor_reduce(out=m_sb[:], in_=lg_sb[:],
                                axis=mybir.AxisListType.X, op=AX.max)
        nm_sb = epool.tile([G, 1], F32, tag="nm")
        nc.vector.tensor_scalar(out=nm_sb[:], in0=m_sb[:], scalar1=-1.0,
                                scalar2=None, op0=AX.mult)
        e_sb = epool.tile([G, NCLS], F32, tag="esm")
        ss_sb = epool.tile([G, 1], F32, tag="ss")
        nc.scalar.activation(out=e_sb[:], in_=lg_sb[:],
                             func=mybir.ActivationFunctionType.Exp,
                             bias=nm_sb[:, 0:1], accum_out=ss_sb[:, 0:1])
        ls_sb = epool.tile([G, 1], F32, tag="ls")
        nc.scalar.activation(out=ls_sb[:], in_=ss_sb[:],
                             func=mybir.ActivationFunctionType.Ln)
        lsm_sb = epool.tile([G, NCLS], F32, tag="lsm")
        nc.vector.tensor_scalar(out=lsm_sb[:], in0=lg_sb[:],
                                scalar1=m_sb[:, 0:1], scalar2=ls_sb[:, 0:1],
                                op0=AX.subtract, op1=AX.subtract)

        nc.sync.dma_start(out=d_out[0:G, :], in_=lsm_sb[:])
        nc.sync.dma_start(out=d_out[G:2 * G, :], in_=lg_sb[:])

    nc.compile()  # bacc register allocation / DCE / act-table loads
    # The module is immutable from here on; memoize its (deterministic)
    # serialization so jit lowering skips the re-serialization.
    _json = nc.to_json_bytes()
    nc.to_json_bytes = lambda: _json
    return nc


class _Exec:
    """One-time-built PJRT callable for the SPMD kernel.  A steady-state
    call is exactly: concat per-core inputs, H2D, NEFF exec, D2H."""

    def __init__(self, meta):
        nc = build_nc(meta)
        install_neuronx_cc_hook()
        partition_name = (nc.partition_id_tensor.name
                          if nc.partition_id_tensor else None)
        in_names, out_names, out_avals = [], [], []
        for alloc in nc.m.functions[0].allocations:
            if not isinstance(alloc, mybir.MemoryLocationSet):
                continue
            name = alloc.memorylocations[0].name
            if alloc.kind == "ExternalInput":
                if name != partition_name:
                    in_names.append(name)
            elif alloc.kind == "ExternalOutput":
                out_names.append(name)
                out_avals.append(jax.core.ShapedArray(
                    tuple(alloc.tensor_shape), mybir.dt.np(alloc.dtype)))
        n_params = len(in_names)
        in_names_all = in_names + out_names
        if partition_name is not None:
            in_names_all.append(partition_name)

        def _body(*args):
            operands = list(args)
            if partition_name is not None:
                operands.append(partition_id_tensor())
            return tuple(_bass_exec_p.bind(
                *operands, out_avals=tuple(out_avals),
                in_names=tuple(in_names_all), out_names=tuple(out_names),
                lowering_input_output_aliases=(), sim_require_finite=True,
                sim_require_nnan=True, nc=nc))

        devices = jax.devices()[:NCORES]
        mesh = Mesh(np.asarray(devices), ("core",))
        n_outs = len(out_names)
        self._fn = jax.jit(
            shard_map(_body, mesh=mesh,
                      in_specs=(PartitionSpec("core"),) * (n_params + n_outs),
                      out_specs=(PartitionSpec("core"),) * n_outs,
                      check_rep=False),
            keep_unused=True)
        self.in_names = in_names
        self.out_names = out_names
        self.out_avals = out_avals
        # the zero "output seed" buffers never change: keep them device-
        # resident (inputs are immutable without donation, so reuse is safe)
        sh = NamedSharding(mesh, PartitionSpec("core"))
        self._zeros = [
            jax.device_put(
                np.zeros((NCORES * a.shape[0], *a.shape[1:]), a.dtype), sh)
            for a in out_avals]

    def __call__(self, in_maps):
        concat_in = [
            np.concatenate([np.asarray(m[name]) for m in in_maps], axis=0)
            for name in self.in_names]
        outs = self._fn(*concat_in, *self._zeros)
        # every core computes the identical pooled result; fetch only core
        # 0's shard (a full np.asarray gather pulls 8 shards through the
        # tunnel with per-shard protocol legs)
        return {name: np.asarray(outs[i].addressable_shards[0].data)
                for i, name in enumerate(self.out_names)}


_EXEC_CACHE = {}


def get_exec(meta):
    key = (meta["N"], meta["E"], meta["NT"], meta["U"], meta["TBS"],
           meta["bias1"], meta["bias2"], meta["lbias1"], meta["lbias2"])
    if key not in _EXEC_CACHE:
        _EXEC_CACHE[key] = _Exec(meta)
    return _EXEC_CACHE[key]


def run_gat(inputs, cfg):
    meta, in_maps = host_prep(inputs, cfg)
    ex = get_exec(meta)
    results = ex(in_maps)
    G, NCLS = cfg["G"], cfg["NCLS"]
    out = results["out"]
    return (out[0:G, :], out[G:2 * G, :]), (ex, in_maps)


def kernel(**inputs):
    (lsm, logits), _ = run_gat(inputs, gat_config())
    return lsm.astype(np.float32), logits.astype(np.float32)
